# revision 1
# baseline (speedup 1.0000x reference)
"""Trainium2 Bass kernel for nn_MixedAttnHeadEmbed (mixed-head-config attention).

Math (per batch b):
  Two attention configs share q_m/k_m/v_m [B,T,2048]:
    A: h=8  heads, d_max=256, mixing e in {1024,2048} -> d in {128,256}, weights w0,w1
    B: h=16 heads, d_max=128, mixing e in {1024,2048} -> d in {64,128},  weights w2,w3
  Each config: per-head q/k slices are RoPE'd, weight-summed (padded to d_max),
  GQA (8 kv heads), causal softmax attention; outputs of both configs sum.

Sharding: 8 cores = 4 batches x 2 shards. Shard s owns A-heads [4s,4s+4) and
B-heads [8s,8s+8) -> both write output columns [1024s, 1024s+1024) which are
summed on device; per-core output is the transposed block outT [1024, T].

Device layout: scores computed transposed (sT[k,q], k on partitions) so the
softmax'd weights feed the y^T matmul with no on-chip transposes; softmax is
max-free (scores are provably < 2 for this problem family; exp is safe in
fp32) with the denominator from an all-ones stationary matmul.
"""

import math
from contextlib import ExitStack
from dataclasses import dataclass

import numpy as np

import concourse.bass as bass
import concourse.mybir as mybir
import concourse.tile as tile
from concourse import bacc

F32 = mybir.dt.float32
F32R = mybir.dt.float32r
NEG = -1e9
P = 128


@dataclass(frozen=True)
class KCfg:
    T: int = 1024       # sequence length
    NA: int = 4         # config-A heads per core (d_max=256)
    NB: int = 8         # config-B heads per core (d_max=128); must be 2*NA
    REG: int = 512      # psum region width (<=512)

    @property
    def TK(self):
        return self.T // P

    @property
    def NREG(self):
        return self.T // self.REG

    @property
    def NKVB(self):
        return self.NB // 2

    @property
    def ROWS(self):
        return self.NA * 256  # == NB * 128 output rows per core


FULL = KCfg()


def _in_specs(cfg: KCfg):
    T = cfg.T
    return {
        "qT1": (cfg.NA * 128, T),
        "qT2": (cfg.NA * 256, T),
        "kTa1": (cfg.NA * 128, T),
        "kTa2": (cfg.NA * 256, T),
        "kTb1": (cfg.NKVB * 64, T),
        "kTb2": (cfg.NKVB * 128, T),
        "va1": (T, cfg.NA * 128),
        "va2": (T, cfg.NA * 256),
        "vb1": (T, cfg.NKVB * 64),
        "vb2": (T, cfg.NKVB * 128),
        "ca1": (128, T), "sa1": (128, T),
        "ca2": (256, T), "sa2": (256, T),
        "cb1": (128, T), "sb1": (128, T),
        "cb2": (128, T), "sb2": (128, T),
        "wvec": (P, 4),
    }


class _EngPick:
    """Static load balancer across DVE / GPSIMD / ACT.

    units: 1.0 ~ one [.,1024] fp32 pass. Cost-model calibration: DVE and
    Pool run TT at ~1 elem/lane/cycle (fp32 has no DVE fast mode); ACT can
    only take single-input copies, and it also carries all the exps (those
    are tallied in via act())."""

    GP_W = 1.05   # tuned: bias work toward pool
    ACT_W = 1.5

    def __init__(self, nc):
        self.nc = nc
        self.load = {"dve": 0.0, "pool": 0.0, "act": 0.0}

    def dve(self, units=1.0):
        self.load["dve"] += units
        return self.nc.vector

    def act(self, units=1.0):
        self.load["act"] += units * self.ACT_W
        return self.nc.scalar

    def tt(self, units=1.0):
        """2-input sbuf op: DVE or GPSIMD."""
        if self.load["dve"] + units <= self.load["pool"] + self.GP_W * units:
            return self.dve(units)
        self.load["pool"] += self.GP_W * units
        return self.nc.gpsimd

    def copy(self, dst, src, units=1.0):
        """1-input copy: any of the three engines."""
        costs = {"dve": units, "pool": self.GP_W * units,
                 "act": self.ACT_W * units}
        eng = min(costs, key=lambda k: self.load[k] + costs[k])
        self.load[eng] += costs[eng]
        if eng == "act":
            self.nc.scalar.copy(dst, src)
        elif eng == "pool":
            self.nc.gpsimd.tensor_copy(dst, src)
        else:
            self.nc.vector.tensor_copy(dst, src)


def build_program(cfg: KCfg = FULL):
    # Bacc (not plain Bass): its compile() runs generate_event_semaphores,
    # which splits multi-wait sync_infos — TRN2 allows 1 wait per instruction.
    nc = bacc.Bacc("TRN2", target_bir_lowering=False)
    T, TK, REG, NREG = cfg.T, cfg.TK, cfg.REG, cfg.NREG
    RPB = REG // P  # k-chunks per region

    D = {}
    for name, shape in _in_specs(cfg).items():
        D[name] = nc.declare_dram_parameter(name, list(shape), F32, isOutput=False)
    outT = nc.declare_dram_parameter("outT", [cfg.ROWS, T], F32, isOutput=True)
    RB = cfg.ROWS // P

    mult, add = mybir.AluOpType.mult, mybir.AluOpType.add

    with ExitStack() as ctx:
        tc = ctx.enter_context(tile.TileContext(nc))
        const = ctx.enter_context(tc.tile_pool(name="const", bufs=1))
        rawp = ctx.enter_context(tc.tile_pool(name="raw", bufs=2))
        mixp = ctx.enter_context(tc.tile_pool(name="mix", bufs=2))
        scr = ctx.enter_context(tc.tile_pool(name="scr", bufs=1))
        ppool = ctx.enter_context(tc.tile_pool(name="pp", bufs=3))
        accp = ctx.enter_context(tc.tile_pool(name="acc", bufs=1))
        normp = ctx.enter_context(tc.tile_pool(name="norm", bufs=1))
        spsum = ctx.enter_context(tc.tile_pool(name="spsum", bufs=2, space="PSUM"))
        ypsum = ctx.enter_context(tc.tile_pool(name="ypsum", bufs=1, space="PSUM"))
        dpsum = ctx.enter_context(tc.tile_pool(name="dpsum", bufs=1, space="PSUM"))

        pick = _EngPick(nc)

        # ---- constants ----
        ones_f = const.tile([P, P], F32, name="ones_f")
        nc.vector.memset(ones_f, 1.0)
        ones = const.tile([P, P], F32R)
        nc.vector.tensor_copy(ones, ones_f)  # rounds to f32r for the matmul
        dmask = const.tile([P, P], F32)
        nc.gpsimd.memset(dmask, 0.0)
        # dmask[k, q] = 0 where q >= k else NEG  (transposed causal diag block)
        nc.gpsimd.affine_select(
            out=dmask, in_=dmask, compare_op=mybir.AluOpType.is_ge,
            fill=NEG, base=0, pattern=[[1, P]], channel_multiplier=-1,
        )
        tabs = {}
        for nm in ("ca1", "sa1", "ca2", "sa2", "cb1", "sb1", "cb2", "sb2"):
            rows = _in_specs(cfg)[nm][0]
            tl = const.tile([P, rows // P, T], F32, name=nm, tag=nm)
            tabs[nm] = tl
            nc.sync.dma_start(out=tl, in_=D[nm].rearrange("(c p) t -> p c t", p=P))
        wv = const.tile([P, 4], F32)
        nc.sync.dma_start(out=wv, in_=D["wvec"][:, :])

        outacc = accp.tile([P, RB, T], F32)

        def halfmul(dst, src, tab, half, base=0, rows=P):
            """dst[base:base+rows][j] = src[sigma(j)] * tab_math[j], where
            sigma swaps halves of size `half` within each 2*half group.

            tab is the HOST-SIGMA-PERMUTED signed sin table, so the multiply
            is same-base (u = src*tab) and the rotation becomes 1-input
            cross-base copies (the only cross-partition-base op trn2 allows).
            """
            u = scr.tile([P, T], F32, tag="xbt", name="xbt")
            usl = u[base:base + rows, :]
            pick.tt(1.0).tensor_tensor(usl, src, tab, mult)
            for g in range(rows // (2 * half)):
                b0 = base + 2 * half * g
                pick.copy(dst[b0:b0 + half, :], u[b0 + half:b0 + 2 * half, :], 1.0)
                pick.copy(dst[b0 + half:b0 + 2 * half, :], u[b0:b0 + half, :], 1.0)

        def xb_add(dst, src, units):
            """dst += src across partition bases (copy to re-base, then add)."""
            n = src.shape[0]
            tmp = scr.tile([P, T], F32, tag="xbt2", name="xbt2")
            view = tmp[0:n, :]
            pick.copy(view, src, units)
            pick.tt(units).tensor_tensor(dst, dst, view, add)

        def mix_qk_A(out, x1, x2, c1, s1, c2, s2):
            """out [P,2,T] = rope-mix for a config-A head.
            x1 [P,T] (d=128 slice), x2 [P,2,T] (d=256 slice).
            s1 is sigma64-permuted; s2 is the plain signed sin table."""
            t1 = scr.tile([P, T], F32, tag="t1")
            t2 = scr.tile([P, T], F32, tag="t2")
            # dc0: x2t0*c2_0 + x2t1*s2_0 + x1*c1 + shift64(x1)*s1
            pick.tt().tensor_tensor(out[:, 0, :], x2[:, 0, :], c2[:, 0, :], mult)
            pick.tt().tensor_tensor(t1, x2[:, 1, :], s2[:, 0, :], mult)
            pick.tt().tensor_tensor(out[:, 0, :], out[:, 0, :], t1, add)
            pick.tt().tensor_tensor(t1, x1, c1[:, 0, :], mult)
            halfmul(t2, x1, s1[:, 0, :], 64)
            pick.tt().tensor_tensor(t1, t1, t2, add)
            pick.tt().tensor_tensor(out[:, 0, :], out[:, 0, :], t1, add)
            # dc1: x2t1*c2_1 + x2t0*s2_1
            pick.tt().tensor_tensor(out[:, 1, :], x2[:, 1, :], c2[:, 1, :], mult)
            pick.tt().tensor_tensor(t1, x2[:, 0, :], s2[:, 1, :], mult)
            pick.tt().tensor_tensor(out[:, 1, :], out[:, 1, :], t1, add)

        def mix_qk_B_pair(out, x1p, x2p, c1, s1, c2, s2):
            """out [P,2,T]: B-head pair. out[:,j,:] for heads (2p+j).
            x2p [P,2,T] (d=128 per head), x1p [P,T] packed pair (d=64 each).
            s2 sigma64-permuted; s1 sigma32-permuted."""
            t1 = scr.tile([P, T], F32, tag="t1")
            t2 = scr.tile([P, T], F32, tag="t2")
            for j in range(2):
                pick.tt().tensor_tensor(out[:, j, :], x2p[:, j, :], c2[:, 0, :], mult)
                halfmul(t1, x2p[:, j, :], s2[:, 0, :], 64)
                pick.tt().tensor_tensor(out[:, j, :], out[:, j, :], t1, add)
            # packed d=64 contributions for both heads of the pair
            pick.tt().tensor_tensor(t1, x1p, c1[:, 0, :], mult)
            halfmul(t2, x1p, s1[:, 0, :], 32)
            pick.tt().tensor_tensor(t1, t1, t2, add)
            pick.tt(1.0).tensor_tensor(out[0:64, 0, :], out[0:64, 0, :],
                                       t1[0:64, :], add)
            xb_add(out[0:64, 1, :], t1[64:128, :], 1.0)

        def subchunks(c):
            out = []
            for r in range(NREG):
                q0 = max(REG * r, P * c)
                q1 = REG * (r + 1)
                if q1 > q0:
                    out.append((r, q0, q1 - q0))
            return out

        def attn_head(qmixs, kmixs, vmix, blks, is_b):
            """qmixs/kmixs: per-d-chunk [P, T] APs; vmix [P, TK, ndc*P].

            Matmul operands are bitcast to float32r: full-rate PE streaming
            (fp32 proper runs at 1/4 rate) with near-fp32 accumulation."""
            ndc = len(qmixs)
            den = dpsum.tile([P, T], F32, tag="den")
            yts = [ypsum.tile([P, T], F32, tag=f"yt{i}", name=f"yt{i}")
                   for i in range(ndc)]
            for c in range(TK):
                for (r, q0, n) in subchunks(c):
                    last_c = min(TK, RPB * (r + 1)) - 1
                    sT = spsum.tile([P, REG], F32, tag="sT")
                    for dc in range(ndc):
                        nc.tensor.matmul(
                            sT[:, :n],
                            kmixs[dc][:, P * c:P * (c + 1)],
                            qmixs[dc][:, q0:q0 + n],
                            start=(dc == 0), stop=(dc == ndc - 1))
                    if q0 == P * c:  # diagonal block gets the causal mask
                        pick.dve(0.125).tensor_tensor(sT[:, :P], sT[:, :P],
                                                      dmask, add)
                    pt = ppool.tile([P, REG], F32R, tag="pT")
                    pick.act(n / 1024.0).activation(
                        pt[:, :n], sT[:, :n], mybir.ActivationFunctionType.Exp)
                    for dc in range(ndc):
                        nc.tensor.matmul(
                            yts[dc][:, q0:q0 + n],
                            vmix[:, c, P * dc:P * (dc + 1)],
                            pt[:, :n],
                            start=(c == 0), stop=(c == last_c))
                    nc.tensor.matmul(den[:, q0:q0 + n], ones,
                                     pt[:, :n],
                                     start=(c == 0), stop=(c == last_c))
            rec = normp.tile([P, T], F32, tag="rec")
            pick.dve(1.0).reciprocal(rec, den)
            for dc in range(ndc):
                blk = blks[dc]
                if not is_b:
                    pick.dve(1.0).tensor_tensor(outacc[:, blk, :], yts[dc][:, :],
                                                rec, mult)
                else:
                    tmp = normp.tile([P, T], F32, tag="btmp")
                    pick.dve(1.0).tensor_tensor(tmp, yts[dc][:, :], rec, mult)
                    pick.tt(1.0).tensor_tensor(outacc[:, blk, :],
                                               outacc[:, blk, :], tmp, add)
                    nc.sync.dma_start(out=outT[P * blk:P * (blk + 1), :],
                                      in_=outacc[:, blk, :])

        # ================= config A =================
        for h in range(cfg.NA):
            q1 = rawp.tile([P, T], F32, tag="rS")
            nc.sync.dma_start(out=q1, in_=D["qT1"][P * h:P * (h + 1), :])
            q2 = rawp.tile([P, 2, T], F32, tag="rD")
            nc.sync.dma_start(out=q2, in_=D["qT2"][256 * h:256 * (h + 1), :]
                              .rearrange("(c p) t -> p c t", p=P))
            qmix = mixp.tile([P, 2, T], F32R, tag="qmix")
            mix_qk_A(qmix, q1, q2, tabs["ca1"], tabs["sa1"], tabs["ca2"], tabs["sa2"])

            k1 = rawp.tile([P, T], F32, tag="rS")
            nc.sync.dma_start(out=k1, in_=D["kTa1"][P * h:P * (h + 1), :])
            k2 = rawp.tile([P, 2, T], F32, tag="rD")
            nc.sync.dma_start(out=k2, in_=D["kTa2"][256 * h:256 * (h + 1), :]
                              .rearrange("(c p) t -> p c t", p=P))
            kmix = mixp.tile([P, 2, T], F32R, tag="kmix")
            mix_qk_A(kmix, k1, k2, tabs["ca1"], tabs["sa1"], tabs["ca2"], tabs["sa2"])

            v1 = rawp.tile([P, TK, P], F32, tag="rv1")
            nc.sync.dma_start(out=v1, in_=D["va1"][:, P * h:P * (h + 1)]
                              .rearrange("(c p) d -> p c d", p=P))
            v2 = rawp.tile([P, TK, 2 * P], F32, tag="rv2")
            nc.sync.dma_start(out=v2, in_=D["va2"][:, 2 * P * h:2 * P * (h + 1)]
                              .rearrange("(c p) d -> p c d", p=P))
            vmix = mixp.tile([P, TK, 2 * P], F32R, tag="vmix")
            pick.dve(2.0).tensor_scalar_mul(vmix, v2, wv[:, 1:2])
            pick.dve(1.0).scalar_tensor_tensor(
                out=vmix[:, :, 0:P], in0=v1, scalar=wv[:, 0:1],
                in1=vmix[:, :, 0:P], op0=mult, op1=add)

            attn_head([qmix[:, 0, :], qmix[:, 1, :]],
                      [kmix[:, 0, :], kmix[:, 1, :]],
                      vmix, (2 * h, 2 * h + 1), is_b=False)

        # ================= config B =================
        for j in range(cfg.NKVB):  # kv head j serves B-heads (2j, 2j+1)
            k2 = rawp.tile([P, T], F32, tag="rS")
            nc.sync.dma_start(out=k2, in_=D["kTb2"][P * j:P * (j + 1), :])
            # packed pair of d=64 kv slices: kv (2*(j//2)), (2*(j//2)+1)
            k1p = rawp.tile([P, T], F32, tag="rS")
            jp = j // 2
            nc.sync.dma_start(out=k1p, in_=D["kTb1"][P * jp:P * (jp + 1), :])

            kmix = mixp.tile([P, T], F32R, tag="kmix")
            t1 = scr.tile([P, T], F32, tag="t1")
            pick.tt().tensor_tensor(kmix, k2, tabs["cb2"][:, 0, :], mult)
            halfmul(t1, k2, tabs["sb2"][:, 0, :], 64)
            pick.tt().tensor_tensor(kmix, kmix, t1, add)
            # d=64 part only on rows 0:64 (uses half of the packed pair tile)
            half = 0 if j % 2 == 0 else 64
            sl = slice(half, half + 64)
            ts = scr.tile([P, T], F32, tag="t2", name="ts")
            pick.tt().tensor_tensor(ts[sl, :], k1p[sl, :],
                                    tabs["cb1"][sl, 0, :], mult)
            tb = scr.tile([P, T], F32, tag="t3", name="tb")
            halfmul(tb, k1p[sl, :], tabs["sb1"][sl, 0, :], 32, base=half, rows=64)
            pick.tt().tensor_tensor(ts[sl, :], ts[sl, :], tb[sl, :], add)
            if half == 0:
                pick.tt().tensor_tensor(kmix[0:64, :], kmix[0:64, :], ts[sl, :], add)
            else:
                xb_add(kmix[0:64, :], ts[sl, :], 1.0)

            v2 = rawp.tile([P, TK, P], F32, tag="rv1")
            nc.sync.dma_start(out=v2, in_=D["vb2"][:, P * j:P * (j + 1)]
                              .rearrange("(c p) d -> p c d", p=P))
            v1 = rawp.tile([P, TK, 64], F32, tag="rv2")
            nc.sync.dma_start(out=v1, in_=D["vb1"][:, 64 * j:64 * (j + 1)]
                              .rearrange("(c p) d -> p c d", p=P))
            vmix = mixp.tile([P, TK, P], F32R, tag="vmix")
            pick.dve(1.0).tensor_scalar_mul(vmix, v2, wv[:, 3:4])
            pick.dve(0.5).scalar_tensor_tensor(
                out=vmix[:, :, 0:64], in0=v1, scalar=wv[:, 2:3],
                in1=vmix[:, :, 0:64], op0=mult, op1=add)

            # q pair for heads (2j, 2j+1)
            q2p = rawp.tile([P, 2, T], F32, tag="rD")
            nc.sync.dma_start(out=q2p, in_=D["qT2"][256 * j:256 * (j + 1), :]
                              .rearrange("(c p) t -> p c t", p=P))
            q1p = rawp.tile([P, T], F32, tag="rS")
            nc.sync.dma_start(out=q1p, in_=D["qT1"][P * j:P * (j + 1), :])
            qmixp = mixp.tile([P, 2, T], F32R, tag="qmix")
            mix_qk_B_pair(qmixp, q1p, q2p, tabs["cb1"], tabs["sb1"],
                          tabs["cb2"], tabs["sb2"])

            for hh in range(2):
                b = 2 * j + hh
                attn_head([qmixp[:, hh, :]], [kmix], vmix, (b,), is_b=True)

    nc.compile()
    return nc


# ---------------------------------------------------------------------------
# Host side
# ---------------------------------------------------------------------------

def _rope_tab(pos, d, f):
    """Transposed rope tables [d, T]: (f*cos, +-f*sin with rot sign folded)."""
    inv = 1.0 / (10000.0 ** (np.arange(0, d, 2, dtype=np.float32) / d))
    ang = inv[:, None] * pos[None, :].astype(np.float32)      # [d/2, T]
    ang = np.concatenate([ang, ang], 0)                        # [d, T]
    c = (f * np.cos(ang)).astype(np.float32)
    s = (f * np.sin(ang)).astype(np.float32)
    s[: d // 2] *= -1.0
    return c, s


def make_core_inputs(q, k, v, pos, weights, s, cfg: KCfg = FULL):
    """q,k,v: [T, 2048] for one batch; returns the per-core input dict."""
    T = cfg.T
    c = np.ascontiguousarray
    arrs = {
        "qT1": c(q[:, 512 * s:512 * s + 512].T),
        "qT2": c(q[:, 1024 * s:1024 * s + 1024].T),
        "kTa1": c(k[:, 512 * s:512 * s + 512].T),
        "kTa2": c(k[:, 1024 * s:1024 * s + 1024].T),
        "kTb1": c(k[:, 256 * s:256 * s + 256].T),
        "kTb2": c(k[:, 512 * s:512 * s + 512].T),
        "va1": c(v[:, 512 * s:512 * s + 512]),
        "va2": c(v[:, 1024 * s:1024 * s + 1024]),
        "vb1": c(v[:, 256 * s:256 * s + 256]),
        "vb2": c(v[:, 512 * s:512 * s + 512]),
    }
    fA = math.sqrt(1.0 / 16.0)
    fB = math.sqrt(1.0 / math.sqrt(128.0))
    ca1, sa1 = _rope_tab(pos, 128, fA * float(weights[0]))
    ca2, sa2 = _rope_tab(pos, 256, fA * float(weights[1]))
    cb1h, sb1h = _rope_tab(pos, 64, fB * float(weights[2]))
    cb2, sb2 = _rope_tab(pos, 128, fB * float(weights[3]))

    def sigma(tab, half):
        # swap halves of size `half` within each 2*half row group
        out = tab.reshape(-1, 2, half, tab.shape[-1])
        return np.ascontiguousarray(
            out[:, ::-1].reshape(tab.shape))

    sb1 = np.vstack([sb1h, sb1h])
    arrs.update({
        # sin tables used through within-tile rotations are stored
        # sigma-permuted (device computes u = x * s_sigma, then rotates u
        # via cross-base copies); sa2 (d=256) rotates across tiles and
        # stays in math order.
        "ca1": ca1, "sa1": sigma(sa1, 64), "ca2": ca2, "sa2": sa2,
        "cb1": np.vstack([cb1h, cb1h]), "sb1": sigma(sb1, 32),
        "cb2": cb2, "sb2": sigma(sb2, 64),
        "wvec": np.tile(np.asarray(weights, np.float32)[None, :], (P, 1)),
        # math-order copies for numpy models (not used by the device)
        "_m_sa1": sa1, "_m_sb1": sb1, "_m_sb2": sb2,
    })
    return arrs


_PROGRAM_CACHE = {}
TRACE = False
LAST_RESULT = None


def kernel(q_m, k_m, v_m, weights, attention_mask, position_ids):
    global LAST_RESULT
    from concourse.bass_utils import run_bass_kernel_spmd

    cfg = FULL
    q_m = np.asarray(q_m, np.float32)
    k_m = np.asarray(k_m, np.float32)
    v_m = np.asarray(v_m, np.float32)
    weights = np.asarray(weights, np.float32)
    attention_mask = np.asarray(attention_mask, np.float32)
    position_ids = np.asarray(position_ids)
    B, T, H = q_m.shape

    # the device program hardcodes the causal structure; verify it holds
    causal = np.where(np.tril(np.ones((T, T), bool)), 0.0, NEG).astype(np.float32)
    for b in range(B):
        assert np.array_equal(attention_mask[b, 0], causal), "non-causal mask"

    if "nc" not in _PROGRAM_CACHE:
        _PROGRAM_CACHE["nc"] = build_program(cfg)
    nc = _PROGRAM_CACHE["nc"]

    in_maps = []
    for b in range(B):
        for s in range(2):
            in_maps.append(make_core_inputs(
                q_m[b], k_m[b], v_m[b], position_ids[b], weights, s, cfg))
    res = run_bass_kernel_spmd(nc, in_maps, list(range(8)), trace=TRACE)
    LAST_RESULT = res
    out = np.zeros((B, T, H), np.float32)
    for b in range(B):
        for s in range(2):
            out[b, :, 1024 * s:1024 * s + 1024] = res.results[2 * b + s]["outT"].T
    return out



# revision 65
# speedup vs baseline: 1.7543x; 1.7543x over previous
"""Trainium2 Bass kernel for nn_MixedAttnHeadEmbed (mixed-head-config attention).

Math (per batch b):
  Two attention configs share q_m/k_m/v_m [B,T,2048]:
    A: h=8  heads, d_max=256, mixing e in {1024,2048} -> d in {128,256}, weights w0,w1
    B: h=16 heads, d_max=128, mixing e in {1024,2048} -> d in {64,128},  weights w2,w3
  Each config: per-head q/k slices are RoPE'd, weight-summed (padded to d_max),
  GQA (8 kv heads), causal softmax attention; outputs of both configs sum.

Sharding: 8 cores = 4 batches x 2 shards. Shard s owns A-heads [4s,4s+4) and
B-heads [8s,8s+8) -> both write output columns [1024s, 1024s+1024) which are
summed on device; per-core output is the transposed block outT [1024, T] fp16.

Device design (driven by the CoreSim cost model):
  * All on-device data is fp16 (PSUM accumulation stays fp32): DVE gets the
    2x fast mode for 2-byte dtypes, the PE runs 1 col/cycle at any moving
    width (f32r pays 4x under 256 cols), and DMA bytes halve. fp16's 5e-4
    epsilon keeps the end-to-end error ~1e-3, far under the 2e-2 gate.
  * RoPE rotations are eliminated on device: the host uploads sigma-permuted
    row copies of each q/k slice (rows swapped within each rotation group),
    so rope+mix is a chain of partition-aligned tensor_tensor ops against
    sign-folded sin/cos tables (weights and 1/sqrt(d) folded in on host).
  * Scores are computed transposed (sT[k,q], k on partitions) so softmax'd
    weights feed the y^T matmul with no transposes; softmax is max-free
    (scores provably < 2) with the denominator from an all-ones matmul.
  * Causal diag-block masking zeroes pt after the exp via a Pool
    affine_select instead of adding a mask into PSUM on DVE.
  * The PE stream is software-pipelined: scores for k-chunk c+1 are issued
    before y/den for chunk c, so the PE does not sit behind each exp.
"""

import math
from contextlib import ExitStack
from dataclasses import dataclass

import numpy as np

import concourse.bass as bass
import concourse.mybir as mybir
import concourse.tile as tile
from concourse import bacc

F32 = mybir.dt.float32
PTA_BUFS, PTB_BUFS, LAG_A, LAG_B = 8, 6, 3, 4
F16 = mybir.dt.float16
P = 128


@dataclass(frozen=True)
class KCfg:
    T: int = 1024       # sequence length
    NA: int = 4         # config-A heads per core (d_max=256)
    NB: int = 8         # config-B heads per core (d_max=128)
    REG: int = 512      # psum region width

    @property
    def TK(self):
        return self.T // P

    @property
    def NREG(self):
        return self.T // self.REG


FULL = KCfg()


def _in_specs(cfg: KCfg):
    T = cfg.T
    return {
        # q/k transposed slices (rows = head dims, fp16). *s = sigma-permuted
        # rows (rotation pairing partner), so rope is all aligned TT ops.
        "qA1": (512, T), "qB1s": (512, T),
        "qA2": (1024, T), "qB2s": (1024, T),
        "kA1": (512, T),
        "kA2": (1024, T),
        "kB1": (256, T), "kB1s": (256, T),
        # v slices, natural [T, d] layout
        "vA1": (T, 512), "vA2": (T, 1024), "vB1": (T, 256),
        # rope tables [d, T], weights+scale folded, sin sign-folded
        "ca1": (128, T), "sa1": (128, T),
        "ca2": (256, T), "sa2": (256, T),
        "cb1": (128, T), "sb1": (128, T),
        "cb2": (128, T), "sb2": (128, T),
        # v mixing: ratio rows (w0/w1, w2/w3) and 1/w den-ones columns
        "wr0": (P, 512), "wr2": (P, 256),
        "onesA": (P, 128), "onesB": (P, 128),
    }


class _Pick:
    """Static DVE-vs-Pool load balancer with cost-model-accurate weights.

    DVE: n*0.5208ns fp16 TT (2x mode), n*0.26 fp16 copy (4x), n*1.0417
    for psum/fp32 ops (+60/+125ns access). Pool: n*0.8333 flat. ACT is
    reserved for the exps (it is the 2nd-busiest engine)."""

    def __init__(self, nc):
        self.nc = nc
        self.load = {"dve": 0.0, "pool": 0.0}

    def _eng(self, cd, cp):
        if self.load["dve"] + cd <= self.load["pool"] + cp:
            self.load["dve"] += cd
            return self.nc.vector
        self.load["pool"] += cp
        return self.nc.gpsimd

    def tt16(self, out, in0, in1, op, n):
        e = self._eng(n * 0.5208 + 60, n * 0.8333 + 25)
        e.tensor_tensor(out, in0, in1, op)

    def recip(self, out, in_, n):
        """PSUM->SBUF reciprocal: DVE only (Pool has no PSUM port)."""
        self.load["dve"] += n * 1.0417 + 125
        self.nc.vector.reciprocal(out, in_)

    def act_exp(self, out, in_, n):
        self.load["act"] = self.load.get("act", 0.0) + n * 0.8333 + 185
        self.nc.scalar.activation(out, in_, mybir.ActivationFunctionType.Exp)

    def copy_ps(self, dst, src, n):
        """PSUM->SBUF copy: ACT ('copy' shares the exp table, no reload)
        or DVE, whichever is less loaded."""
        ca = n * 0.8333 + 185
        cd = n * 1.0417 + 125
        if self.load.get("act", 0.0) + ca <= self.load["dve"] + cd:
            self.load["act"] = self.load.get("act", 0.0) + ca
            self.nc.scalar.copy(dst, src)
        else:
            self.load["dve"] += cd
            self.nc.vector.tensor_copy(dst, src)

    def copy16(self, dst, src, n):
        e = self._eng(n * 0.26 + 60, n * 0.8333 + 25)
        e.tensor_copy(dst, src)

    def pool_fix(self, n):
        self.load["pool"] += n * 0.8333 + 25
        return self.nc.gpsimd


def build_program(cfg: KCfg = FULL):
    nc = bacc.Bacc("TRN2", target_bir_lowering=False)
    T, TK, REG, NREG = cfg.T, cfg.TK, cfg.REG, cfg.NREG
    RPB = REG // P

    D = {}
    for name, shape in _in_specs(cfg).items():
        D[name] = nc.declare_dram_parameter(name, list(shape), F16, isOutput=False)
    outT = nc.declare_dram_parameter("outT", [1024, T], F16, isOutput=True)

    mult, add = mybir.AluOpType.mult, mybir.AluOpType.add

    def subchunks(c):
        out = []
        for r in range(NREG):
            q0 = max(REG * r, P * c)
            q1 = REG * (r + 1)
            if q1 > q0:
                out.append((r, q0, q1 - q0))
        return out

    last_c = [min(TK, RPB * (r + 1)) - 1 for r in range(NREG)]

    with ExitStack() as ctx:
        tc = ctx.enter_context(tile.TileContext(nc))
        const = ctx.enter_context(tc.tile_pool(name="const", bufs=1))
        rawq = ctx.enter_context(tc.tile_pool(name="rawq", bufs=1))
        vmixp = ctx.enter_context(tc.tile_pool(name="vmix", bufs=1))
        vraw = ctx.enter_context(tc.tile_pool(name="vraw", bufs=1))
        accp = ctx.enter_context(tc.tile_pool(name="acc", bufs=1))
        x2p = ctx.enter_context(tc.tile_pool(name="x2p", bufs=2))
        mixp = ctx.enter_context(tc.tile_pool(name="mix", bufs=2))
        scr = ctx.enter_context(tc.tile_pool(name="scr", bufs=2))
        normp = ctx.enter_context(tc.tile_pool(name="norm", bufs=2))
        pta = ctx.enter_context(tc.tile_pool(name="pta", bufs=PTA_BUFS))
        ptb = ctx.enter_context(tc.tile_pool(name="ptb", bufs=PTB_BUFS))

        pick = _Pick(nc)

        # DMAs are emitted in first-use order (the DMA device is serial in
        # the model): A tables -> head-0 slices -> v chunks -> later heads ->
        # B-only tables/slices. Raw q/k persistent tiles are filled by
        # per-head slice DMAs so head 0's data lands first.
        onesA = const.tile([P, P], F16)
        onesB = const.tile([P, P], F16)
        nc.sync.dma_start(out=onesA, in_=D["onesA"][:, :])
        nc.sync.dma_start(out=onesB, in_=D["onesB"][:, :])
        tabs = {}

        def load_tab(nm):
            rows = _in_specs(cfg)[nm][0]
            tl = const.tile([P, rows // P, T], F16, name=nm, tag=nm)
            tabs[nm] = tl
            nc.sync.dma_start(out=tl, in_=D[nm].rearrange("(c p) t -> p c t", p=P))

        wrs = {}

        def load_wrs():
            for nm in ("wr0", "wr2"):
                cols = _in_specs(cfg)[nm][1]
                tl = const.tile([P, cols], F16, name=nm, tag=nm)
                wrs[nm] = tl
                nc.sync.dma_start(out=tl, in_=D[nm][:, :])

        raw = {}
        for nm in ("qA1", "qB1s", "kA1", "kB1", "kB1s"):
            rows = _in_specs(cfg)[nm][0]
            raw[nm] = rawq.tile([P, rows // P, T], F16, name=nm, tag=nm)
        for nm in ("qA1s", "kA1s"):  # device-built sigma64 copies
            raw[nm] = rawq.tile([P, 4, T], F16, name=nm, tag=nm)

        def emit_sigma64(nm, h):
            s_, d_ = raw[nm[:3]], raw[nm]
            pick.copy16(d_[0:64, h, :], s_[64:128, h, :], 1024)
            pick.copy16(d_[64:128, h, :], s_[0:64, h, :], 1024)

        def load_raw_slice(nm, i):
            nc.sync.dma_start(out=raw[nm][:, i, :],
                              in_=D[nm][P * i:P * (i + 1), :])

        def load_x2(dram, pair, tag):
            tl = x2p.tile([P, 2, T], F16, tag=tag, name=tag)
            for i in range(2):  # split halves: part 0 usable earlier
                nc.sync.dma_start(
                    out=tl[:, i, :],
                    in_=dram[P * (2 * pair + i):P * (2 * pair + i + 1), :])
            return tl

        # head-0 loads, interleaved with the tables each mix op needs so the
        # first products can start ~4us in; wrs (v-mix weights) come last
        q2_0 = x2p.tile([P, 2, T], F16, tag="x2a", name="x2a")
        k2_0 = x2p.tile([P, 2, T], F16, tag="x2b", name="x2b")
        nc.sync.dma_start(out=q2_0[:, 0, :], in_=D["qA2"][0:P, :])
        load_tab("ca2")
        nc.sync.dma_start(out=q2_0[:, 1, :], in_=D["qA2"][P:2 * P, :])
        load_tab("sa2")
        nc.sync.dma_start(out=k2_0[:, 0, :], in_=D["kA2"][0:P, :])
        nc.sync.dma_start(out=k2_0[:, 1, :], in_=D["kA2"][P:2 * P, :])
        load_raw_slice("kA1", 0)
        load_raw_slice("qA1", 0)
        load_tab("ca1")
        load_tab("sa1")
        load_wrs()

        # v tiles; vA2 is DMA'd straight into vmixA and scaled in place.
        # Chunk DMAs are interleaved with head-1's loads further down so the
        # serial DMA queue tracks consumption order.
        vmixA = vmixp.tile([P, TK, 4, 256], F16)
        vmixB = vmixp.tile([P, TK, 4, 128], F16)
        vB1r = vraw.tile([P, TK, 256], F16)

        def load_v_chunk(c):
            nc.sync.dma_start(out=vmixA[:, c],
                              in_=D["vA2"][P * c:P * (c + 1), :])
            nc.sync.dma_start(out=vmixB[:, c],
                              in_=D["vA1"][P * c:P * (c + 1), :])

        # pt is softmax-scale-invariant, so w1/w3 live in the den "ones"
        # columns: vmixA = vA2 + (w0/w1)*vA1 pad; vmixB = vA1 + (w2/w3)*vB1.
        # vmixA's ratio-mult reads vmixB (raw vA1) BEFORE emit_vmixB's add
        # modifies its first 64 cols per kv -- emission order guarantees it.

        def emit_vmixA():
            for c in range(TK):
                vt1 = scr.tile([P, 4, 128], F16, tag="mt2")
                pick.tt16(vt1, vmixB[:, c, :], wrs["wr0"], mult, 512)
                pick.tt16(vmixA[:, c, :, 0:128], vmixA[:, c, :, 0:128], vt1,
                          add, 512)

        def emit_vmixB():
            for c in range(TK):
                vt2 = scr.tile([P, 4, 64], F16, tag="mu2")
                pick.tt16(vt2, vB1r[:, c, :], wrs["wr2"], mult, 256)
                pick.tt16(vmixB[:, c, :, 0:64], vmixB[:, c, :, 0:64], vt2,
                          add, 256)

        outacc = accp.tile([P, 8, T], F16)

        def mix_A_dc0(dst, x1, x1s, x2):
            """dst[:,0,:] = (x2_0*c2_0 + x2_1*s2_0) + (x1*c1 + x1s*s1)."""
            t = scr.tile([P, T], F16, tag="mt")
            u = scr.tile([P, T], F16, tag="mu")
            t2 = scr.tile([P, T], F16, tag="mt2")
            u2 = scr.tile([P, T], F16, tag="mu2")
            pick.tt16(t, x2[:, 0, :], tabs["ca2"][:, 0, :], mult, 1024)
            pick.tt16(u, x2[:, 1, :], tabs["sa2"][:, 0, :], mult, 1024)
            pick.tt16(t2, x1, tabs["ca1"][:, 0, :], mult, 1024)
            pick.tt16(u2, x1s, tabs["sa1"][:, 0, :], mult, 1024)
            pick.tt16(t, t, u, add, 1024)
            pick.tt16(t2, t2, u2, add, 1024)
            pick.tt16(dst[:, 0, :], t, t2, add, 1024)

        def mix_A_dc1(dst, x2):
            """dst[:,1,:] = x2_1*c2_1 + x2_0*s2_1."""
            t = scr.tile([P, T], F16, tag="mt")
            u = scr.tile([P, T], F16, tag="mu")
            pick.tt16(t, x2[:, 1, :], tabs["ca2"][:, 1, :], mult, 1024)
            pick.tt16(u, x2[:, 0, :], tabs["sa2"][:, 1, :], mult, 1024)
            pick.tt16(dst[:, 1, :], t, u, add, 1024)

        def mix_B_pair(dst, x2, x2s, x1, x1s):
            """dst [P,2,T]: B-heads (2j,2j+1) rope mix.
            x2/x2s [P,2,T] d128 nat/sigma64; x1/x1s [P,T] packed d64 pair."""
            for hh in range(2):
                t2 = scr.tile([P, T], F16, tag="mt")
                u2 = scr.tile([P, T], F16, tag="mu")
                pick.tt16(t2, x2[:, hh, :], tabs["cb2"][:, 0, :], mult, 1024)
                pick.tt16(u2, x2s[:, hh, :], tabs["sb2"][:, 0, :], mult, 1024)
                pick.tt16(dst[:, hh, :], t2, u2, add, 1024)
            # packed d64 part for both heads
            t = scr.tile([P, T], F16, tag="mt2")
            u = scr.tile([P, T], F16, tag="mu2")
            pick.tt16(t, x1, tabs["cb1"][:, 0, :], mult, 1024)
            pick.tt16(u, x1s, tabs["sb1"][:, 0, :], mult, 1024)
            pick.tt16(t, t, u, add, 1024)
            # head 2j: rows 0:64 aligned; head 2j+1: cross-base copy first
            pick.tt16(dst[0:64, 0, :], dst[0:64, 0, :], t[0:64, :], add, 1024)
            pick.copy16(u[0:64, :], t[64:128, :], 1024)
            pick.tt16(dst[0:64, 1, :], dst[0:64, 1, :], u[0:64, :], add, 1024)

        def attn_head(qmix_dcs, kmix_dcs, vmix_ap, blks, spool, ypool, dpool,
                      wide_sT, is_b, lag=1, mid_emit=None):
            ones = onesB if is_b else onesA
            """qmix_dcs/kmix_dcs: list of [P,T] APs (one per 128-d chunk).
            vmix_ap(c, dc) -> stationary [P,128]. blks: output block ids.
            Emits PE work software-pipelined: scores(c) ... y/den(c-lag)."""
            ndc = len(qmix_dcs)
            den = dpool.tile([P, T], F32, tag="den")
            yts = [ypool.tile([P, T], F32, tag=f"yt{i}", name=f"yt{i}")
                   for i in range(ndc)]
            pending = []  # deferred (c, parts) lists, flushed `lag` behind

            def emit_norm(r):
                # region r's den/y accumulation is complete: normalize and
                # (for B) add+store now, overlapping the remaining chunks.
                # rec stays in fp16's normal range: the host pre-scales v and
                # the ones columns by 1/16 (softmax is pt-scale-invariant).
                sl = slice(REG * r, REG * (r + 1))
                rec = normp.tile([P, REG], F16, tag="rec")
                with nc.allow_low_precision(
                        reason="softmax rec in fp16; host pre-scales 1/16 "
                               "so rec stays in fp16 normal range"):
                    pick.recip(rec, den[:, sl], REG)
                for dc in range(ndc):
                    blk = blks[dc]
                    y_sb = normp.tile([P, REG], F16, tag="ysb")
                    pick.copy_ps(y_sb, yts[dc][:, sl], REG)
                    if not is_b:
                        pick.tt16(outacc[:, blk, sl], y_sb, rec, mult, 512)
                    else:
                        tmp = normp.tile([P, REG], F16, tag="btmp")
                        pick.tt16(tmp, y_sb, rec, mult, 512)
                        pick.tt16(outacc[:, blk, sl], outacc[:, blk, sl],
                                  tmp, add, 512)
                        nc.sync.dma_start(
                            out=outT[P * blk:P * (blk + 1), sl],
                            in_=outacc[:, blk, sl])

            def flush(pend):
                c, parts = pend
                for (q0, n, pt, off) in parts:
                    r = q0 // REG
                    # the diagonal 128 cols wait on the affine mask; split
                    # them off so the bulk matmuls only depend on the exp.
                    # (not at c==0: two start=True groups in one PSUM zero
                    # region are illegal)
                    segs = [(q0, n, off)]
                    if q0 == P * c and n > P and c > 0:
                        segs = [(q0 + P, n - P, off + P), (q0, P, off)]
                    for (s0, sn, so) in segs:
                        for dc in range(ndc):
                            nc.tensor.matmul(
                                yts[dc][:, s0:s0 + sn], vmix_ap(c, dc),
                                pt[:, so:so + sn],
                                start=(c == 0), stop=(c == last_c[r]))
                        nc.tensor.matmul(den[:, s0:s0 + sn], ones,
                                         pt[:, so:so + sn],
                                         start=(c == 0), stop=(c == last_c[r]))
                for r in range(NREG):
                    if last_c[r] == c:
                        emit_norm(r)

            for c in range(TK):
                if c == 3 and mid_emit is not None:
                    mid_emit()
                parts = []
                if wide_sT:
                    # one [P,T] sT + one exp for the whole chunk
                    sT = spool.tile([P, T], F32, tag="sTw")
                    pt = ptb.tile([P, T], F16, tag="ptw")
                    for (r, q0, n) in subchunks(c):
                        for dc in range(ndc):
                            nc.tensor.matmul(
                                sT[:, q0:q0 + n],
                                kmix_dcs[dc][:, P * c:P * (c + 1)],
                                qmix_dcs[dc][:, q0:q0 + n],
                                start=(dc == 0), stop=(dc == ndc - 1))
                    pick.act_exp(pt[:, P * c:], sT[:, P * c:], T - P * c)
                    parts = [(q0, n, pt, q0) for (r, q0, n) in subchunks(c)]
                    diag = (pt, P * c)
                else:
                    # [P,REG] sT + exp per subchunk
                    diag = None
                    for (r, q0, n) in subchunks(c):
                        sT = spool.tile([P, REG], F32, tag="sTn")
                        pt = pta.tile([P, REG], F16, tag="ptn")
                        for dc in range(ndc):
                            nc.tensor.matmul(
                                sT[:, 0:n],
                                kmix_dcs[dc][:, P * c:P * (c + 1)],
                                qmix_dcs[dc][:, q0:q0 + n],
                                start=(dc == 0), stop=(dc == ndc - 1))
                        pick.act_exp(pt[:, 0:n], sT[:, 0:n], n)
                        if q0 == P * c:
                            diag = (pt, 0)
                        parts.append((q0, n, pt, 0))
                # zero the masked (k>q) half of the causal diagonal block
                dpt, doff = diag
                pick.pool_fix(128).affine_select(
                    out=dpt[:, doff:doff + P], in_=dpt[:, doff:doff + P],
                    compare_op=mybir.AluOpType.is_ge, fill=0.0,
                    base=0, pattern=[[1, P]], channel_multiplier=-1)
                pending.append((c, parts))
                if len(pending) > lag:
                    flush(pending.pop(0))
            for pend in pending:
                flush(pend)

        def mix_A_head(h, q2=None, k2=None):
            # DMAs emitted here so the serial DMA queue runs in use order;
            # dc0 chains for q AND k first so the first score matmul (which
            # only needs dc0 of both) is unblocked as early as possible.
            if q2 is None:
                q2 = load_x2(D["qA2"], h, "x2a")
                load_raw_slice("qA1", h)
                k2 = load_x2(D["kA2"], h, "x2b")
                load_raw_slice("kA1", h)
            emit_sigma64("qA1s", h)
            emit_sigma64("kA1s", h)
            qmix = mixp.tile([P, 2, T], F16, tag="qmix")
            kmix = mixp.tile([P, 2, T], F16, tag="kmix")
            mix_A_dc0(qmix, raw["qA1"][:, h, :], raw["qA1s"][:, h, :], q2)
            mix_A_dc0(kmix, raw["kA1"][:, h, :], raw["kA1s"][:, h, :], k2)
            mix_A_dc1(qmix, q2)
            mix_A_dc1(kmix, k2)
            return qmix, kmix

        def mix_B_group(j):
            """kv j's kmix + the q pair for heads (2j, 2j+1)."""
            if j == 0:
                for nm in ("cb2", "sb2", "cb1", "sb1"):
                    load_tab(nm)
            if j % 2 == 0:
                load_raw_slice("kB1", j // 2)
                load_raw_slice("kB1s", j // 2)
            load_raw_slice("qB1s", j)
            kmix = mixp.tile([P, T], F16, tag="kmixB")
            t = scr.tile([P, T], F16, tag="mt2")
            u = scr.tile([P, T], F16, tag="mu2")
            pick.tt16(t, raw["kA1"][:, j, :], tabs["cb2"][:, 0, :], mult, 1024)
            pick.tt16(u, raw["kA1s"][:, j, :], tabs["sb2"][:, 0, :], mult, 1024)
            pick.tt16(kmix, t, u, add, 1024)
            # d64 part: computed once per kv pair (kB1 tile j//2), reused by
            # the odd kv via a cross-base copy
            jp, g = j // 2, j % 2
            if g == 0:
                t64 = scr.tile([P, T], F16, tag="t64")
                pick.tt16(t64, raw["kB1"][:, jp, :], tabs["cb1"][:, 0, :],
                          mult, 1024)
                pick.tt16(u, raw["kB1s"][:, jp, :], tabs["sb1"][:, 0, :],
                          mult, 1024)
                pick.tt16(t64, t64, u, add, 1024)
                st["t64"] = t64
                pick.tt16(kmix[0:64, :], kmix[0:64, :], t64[0:64, :], add, 1024)
            else:
                pick.copy16(u[0:64, :], st["t64"][64:128, :], 1024)
                pick.tt16(kmix[0:64, :], kmix[0:64, :], u[0:64, :], add, 1024)

            q2 = load_x2(D["qA2"], j, "x2a")
            q2s = load_x2(D["qB2s"], j, "x2b")
            qmixp = mixp.tile([P, 2, T], F16, tag="qmix")
            mix_B_pair(qmixp, q2, q2s,
                       raw["qA1"][:, j, :], raw["qB1s"][:, j, :])
            return qmixp, kmix

        # ============ config A (B group 0's mix overlaps A3's attn) ========
        st = {"amix": mix_A_head(0, q2_0, k2_0), "bmix": None}
        # interleave v-chunk DMAs with head 1's loads in the serial DMA queue
        load_v_chunk(0)
        load_v_chunk(1)
        q2_1 = load_x2(D["qA2"], 1, "x2a")
        load_raw_slice("qA1", 1)
        load_v_chunk(2)
        load_v_chunk(3)
        k2_1 = load_x2(D["kA2"], 1, "x2b")
        load_raw_slice("kA1", 1)
        for c in range(4, TK):
            load_v_chunk(c)
        emit_vmixA()

        with tc.tile_pool(name="spsA", bufs=2, space="PSUM") as spsA, \
             tc.tile_pool(name="ypsA", bufs=1, space="PSUM") as ypsA, \
             tc.tile_pool(name="dpsA", bufs=1, space="PSUM") as dpsA:
            for h in range(cfg.NA):
                qmix, kmix = st["amix"]

                def filler(h=h):
                    if h == 0:
                        st["amix"] = mix_A_head(1, q2_1, k2_1)
                        for c in range(TK):
                            nc.sync.dma_start(out=vB1r[:, c, :],
                                              in_=D["vB1"][P * c:P * (c + 1), :])
                    elif h == 1:
                        st["amix"] = mix_A_head(2)
                        emit_vmixB()
                    elif h == 2:
                        st["amix"] = mix_A_head(3)
                    else:
                        st["bmix"] = mix_B_group(0)

                def vap(c, dc, h=h):
                    return vmixA[:, c, h, 128 * dc:128 * (dc + 1)]

                attn_head([qmix[:, 0, :], qmix[:, 1, :]],
                          [kmix[:, 0, :], kmix[:, 1, :]],
                          vap, (2 * h, 2 * h + 1), spsA, ypsA, dpsA,
                          wide_sT=False, is_b=False, lag=LAG_A, mid_emit=filler)

        # ================= config B =================
        with tc.tile_pool(name="spsB", bufs=2, space="PSUM") as spsB, \
             tc.tile_pool(name="ypsB", bufs=1, space="PSUM") as ypsB, \
             tc.tile_pool(name="dpsB", bufs=1, space="PSUM") as dpsB:
            for j in range(4):  # kv head j serves B-heads (2j, 2j+1)
                qmixp, kmix = st["bmix"]

                def vapB(c, dc, j=j):
                    return vmixB[:, c, j, :]

                for hh in range(2):
                    b = 2 * j + hh
                    filler = None
                    if hh == 1 and j + 1 < 4:
                        def filler(j=j):
                            st["bmix"] = mix_B_group(j + 1)
                    attn_head([qmixp[:, hh, :]], [kmix], vapB, (b,),
                              spsB, ypsB, dpsB, wide_sT=True, is_b=True,
                              lag=LAG_B, mid_emit=filler)

    nc.compile()
    return nc


# ---------------------------------------------------------------------------
# Host side
# ---------------------------------------------------------------------------

def _rope_tab(pos, d, f):
    """Rope tables [d, T]: (f*cos, signed f*sin); sign folded so that
    out[j] = x[j]*c[j] + x[sigma(j)]*s[j] with sigma the half-swap."""
    inv = 1.0 / (10000.0 ** (np.arange(0, d, 2, dtype=np.float32) / d))
    ang = inv[:, None] * pos[None, :].astype(np.float32)      # [d/2, T]
    ang = np.concatenate([ang, ang], 0)                        # [d, T]
    c = (f * np.cos(ang)).astype(np.float32)
    s = (f * np.sin(ang)).astype(np.float32)
    s[: d // 2] *= -1.0
    return c, s


def _sigma(x, half, group):
    """Swap half-blocks of `half` rows within each `group`-row group."""
    r = x.reshape(-1, 2, half, x.shape[-1])
    assert group == 2 * half
    return np.ascontiguousarray(r[:, ::-1].reshape(x.shape))


def make_core_inputs(q, k, v, pos, weights, s, cfg: KCfg = FULL):
    """q,k,v: [T, 2048] fp32 for one batch; returns per-core input dict."""
    f16 = lambda a: np.ascontiguousarray(a, dtype=np.float16)
    qT1 = np.ascontiguousarray(q[:, 512 * s:512 * s + 512].T)
    qT2 = np.ascontiguousarray(q[:, 1024 * s:1024 * s + 1024].T)
    kT1 = np.ascontiguousarray(k[:, 512 * s:512 * s + 512].T)
    kT2 = np.ascontiguousarray(k[:, 1024 * s:1024 * s + 1024].T)
    kB1 = np.ascontiguousarray(k[:, 256 * s:256 * s + 256].T)
    arrs = {
        "qA1": f16(qT1),
        "qB1s": f16(_sigma(qT1, 32, 64)),
        "qA2": f16(qT2), "qB2s": f16(_sigma(qT2, 64, 128)),
        "kA1": f16(kT1),
        "kA2": f16(kT2),
        "kB1": f16(kB1), "kB1s": f16(_sigma(kB1, 32, 64)),
        "vA1": f16(v[:, 512 * s:512 * s + 512] / 16.0),
        "vA2": f16(v[:, 1024 * s:1024 * s + 1024] / 16.0),
        "vB1": f16(v[:, 256 * s:256 * s + 256] / 16.0),
    }
    fA = math.sqrt(1.0 / 16.0)
    fB = math.sqrt(1.0 / math.sqrt(128.0))
    ca1, sa1 = _rope_tab(pos, 128, fA * float(weights[0]))
    ca2, sa2 = _rope_tab(pos, 256, fA * float(weights[1]))
    cb1h, sb1h = _rope_tab(pos, 64, fB * float(weights[2]))
    cb2, sb2 = _rope_tab(pos, 128, fB * float(weights[3]))
    arrs.update({
        "ca1": f16(ca1), "sa1": f16(sa1),
        "ca2": f16(ca2), "sa2": f16(sa2),
        "cb1": f16(np.vstack([cb1h, cb1h])), "sb1": f16(np.vstack([sb1h, sb1h])),
        "cb2": f16(cb2), "sb2": f16(sb2),
        # pt is scale-invariant under softmax: fold w1 (w3) into the den
        # "ones" columns and keep only the w0/w1 (w2/w3) ratio on the v1 add
        "wr0": f16(np.full((P, 512), float(weights[0] / weights[1]))),
        "wr2": f16(np.full((P, 256), float(weights[2] / weights[3]))),
        "onesA": f16(np.full((P, 128), float(1.0 / (16.0 * weights[1])))),
        "onesB": f16(np.full((P, 128), float(1.0 / (16.0 * weights[3])))),
    })
    return arrs


_PROGRAM_CACHE = {}
TRACE = False
LAST_RESULT = None
NEG = -1e9


def kernel(q_m, k_m, v_m, weights, attention_mask, position_ids):
    global LAST_RESULT
    from concourse.bass_utils import run_bass_kernel_spmd

    cfg = FULL
    q_m = np.asarray(q_m, np.float32)
    k_m = np.asarray(k_m, np.float32)
    v_m = np.asarray(v_m, np.float32)
    weights = np.asarray(weights, np.float32)
    attention_mask = np.asarray(attention_mask, np.float32)
    position_ids = np.asarray(position_ids)
    B, T, H = q_m.shape

    # the device program hardcodes the causal structure; verify it holds
    causal = np.where(np.tril(np.ones((T, T), bool)), 0.0, NEG).astype(np.float32)
    for b in range(B):
        assert np.array_equal(attention_mask[b, 0], causal), "non-causal mask"

    if "nc" not in _PROGRAM_CACHE:
        _PROGRAM_CACHE["nc"] = build_program(cfg)
    nc = _PROGRAM_CACHE["nc"]

    in_maps = []
    for b in range(B):
        for s in range(2):
            in_maps.append(make_core_inputs(
                q_m[b], k_m[b], v_m[b], position_ids[b], weights, s, cfg))
    res = run_bass_kernel_spmd(nc, in_maps, list(range(8)), trace=TRACE)
    LAST_RESULT = res
    out = np.zeros((B, T, H), np.float32)
    for b in range(B):
        for s in range(2):
            out[b, :, 1024 * s:1024 * s + 1024] = \
                res.results[2 * b + s]["outT"].astype(np.float32).T
    return out


# revision 68
# speedup vs baseline: 1.8128x; 1.0333x over previous
"""Trainium2 Bass kernel for nn_MixedAttnHeadEmbed (mixed-head-config attention).

Math (per batch b):
  Two attention configs share q_m/k_m/v_m [B,T,2048]:
    A: h=8  heads, d_max=256, mixing e in {1024,2048} -> d in {128,256}, weights w0,w1
    B: h=16 heads, d_max=128, mixing e in {1024,2048} -> d in {64,128},  weights w2,w3
  Each config: per-head q/k slices are RoPE'd, weight-summed (padded to d_max),
  GQA (8 kv heads), causal softmax attention; outputs of both configs sum.

Sharding: 8 cores = 4 batches x 2 shards. Shard s owns A-heads [4s,4s+4) and
B-heads [8s,8s+8) -> both write output columns [1024s, 1024s+1024) which are
summed on device; per-core output is the transposed block outT [1024, T] fp16.

Device design (driven by the CoreSim cost model):
  * All on-device data is fp16 (PSUM accumulation stays fp32): DVE gets the
    2x fast mode for 2-byte dtypes, the PE runs 1 col/cycle at any moving
    width (f32r pays 4x under 256 cols), and DMA bytes halve. fp16's 5e-4
    epsilon keeps the end-to-end error ~1e-3, far under the 2e-2 gate.
  * RoPE rotations are eliminated on device: the host uploads sigma-permuted
    row copies of each q/k slice (rows swapped within each rotation group),
    so rope+mix is a chain of partition-aligned tensor_tensor ops against
    sign-folded sin/cos tables (weights and 1/sqrt(d) folded in on host).
  * Scores are computed transposed (sT[k,q], k on partitions) so softmax'd
    weights feed the y^T matmul with no transposes; softmax is max-free
    (scores provably < 2) with the denominator from an all-ones matmul.
  * Causal diag-block masking zeroes pt after the exp via a Pool
    affine_select instead of adding a mask into PSUM on DVE.
  * The PE stream is software-pipelined: scores for k-chunk c+1 are issued
    before y/den for chunk c, so the PE does not sit behind each exp.
"""

import math
from contextlib import ExitStack
from dataclasses import dataclass

import numpy as np

import concourse.bass as bass
import concourse.mybir as mybir
import concourse.tile as tile
from concourse import bacc

F32 = mybir.dt.float32
PTA_BUFS, PTB_BUFS, LAG_A, LAG_B = 10, 7, 4, 5
F16 = mybir.dt.float16
P = 128


@dataclass(frozen=True)
class KCfg:
    T: int = 1024       # sequence length
    NA: int = 4         # config-A heads per core (d_max=256)
    NB: int = 8         # config-B heads per core (d_max=128)
    REG: int = 512      # psum region width

    @property
    def TK(self):
        return self.T // P

    @property
    def NREG(self):
        return self.T // self.REG


FULL = KCfg()


def _in_specs(cfg: KCfg):
    T = cfg.T
    return {
        # q/k transposed slices (rows = head dims, fp16). *s = sigma-permuted
        # rows (rotation pairing partner), so rope is all aligned TT ops.
        "qA1": (512, T), "qB1s": (512, T),
        "qA2": (1024, T), "qB2s": (1024, T),
        "kA1": (512, T),
        "kA2": (1024, T),
        "kB1": (256, T), "kB1s": (256, T),
        # v slices, natural [T, d] layout
        "vA1": (T, 512), "vA2": (T, 1024), "vB1": (T, 256),
        # rope tables [d, T], weights+scale folded, sin sign-folded
        "ca1": (128, T), "sa1": (128, T),
        "ca2": (256, T), "sa2": (256, T),
        "cb1": (128, T), "sb1": (128, T),
        "cb2": (128, T), "sb2": (128, T),
        # v mixing: ratio rows (w0/w1, w2/w3) and 1/w den-ones columns
        "wr0": (P, 512), "wr2": (P, 256),
        "onesA": (P, 128), "onesB": (P, 128),
    }


class _Pick:
    """Static DVE-vs-Pool load balancer with cost-model-accurate weights.

    DVE: n*0.5208ns fp16 TT (2x mode), n*0.26 fp16 copy (4x), n*1.0417
    for psum/fp32 ops (+60/+125ns access). Pool: n*0.8333 flat. ACT is
    reserved for the exps (it is the 2nd-busiest engine)."""

    def __init__(self, nc):
        self.nc = nc
        self.load = {"dve": 0.0, "pool": 0.0}

    def _eng(self, cd, cp):
        if self.load["dve"] + cd <= self.load["pool"] + cp:
            self.load["dve"] += cd
            return self.nc.vector
        self.load["pool"] += cp
        return self.nc.gpsimd

    def tt16(self, out, in0, in1, op, n):
        e = self._eng(n * 0.5208 + 60, n * 0.8333 + 25)
        e.tensor_tensor(out, in0, in1, op)

    def recip(self, out, in_, n):
        """PSUM->SBUF reciprocal: DVE only (Pool has no PSUM port)."""
        self.load["dve"] += n * 1.0417 + 125
        self.nc.vector.reciprocal(out, in_)

    def act_exp(self, out, in_, n):
        self.load["act"] = self.load.get("act", 0.0) + n * 0.8333 + 185
        self.nc.scalar.activation(out, in_, mybir.ActivationFunctionType.Exp)

    def copy_ps(self, dst, src, n, on_act):
        """PSUM->SBUF copy. Phase-aware placement: ACT idles during config A
        (PE-bound) but saturates during config B, where DVE/Pool idle --
        so A-head copies go to ACT ('copy' shares the exp table, no reload)
        and B-head copies go to DVE."""
        if on_act:
            self.load["act"] = self.load.get("act", 0.0) + n * 0.8333 + 185
            self.nc.scalar.copy(dst, src)
        else:
            self.load["dve"] += n * 1.0417 + 125
            self.nc.vector.tensor_copy(dst, src)

    def copy16(self, dst, src, n):
        e = self._eng(n * 0.26 + 60, n * 0.8333 + 25)
        e.tensor_copy(dst, src)

    def pool_fix(self, n):
        self.load["pool"] += n * 0.8333 + 25
        return self.nc.gpsimd


def build_program(cfg: KCfg = FULL):
    nc = bacc.Bacc("TRN2", target_bir_lowering=False)
    T, TK, REG, NREG = cfg.T, cfg.TK, cfg.REG, cfg.NREG
    RPB = REG // P

    D = {}
    for name, shape in _in_specs(cfg).items():
        D[name] = nc.declare_dram_parameter(name, list(shape), F16, isOutput=False)
    outT = nc.declare_dram_parameter("outT", [1024, T], F16, isOutput=True)

    mult, add = mybir.AluOpType.mult, mybir.AluOpType.add

    def subchunks(c):
        out = []
        for r in range(NREG):
            q0 = max(REG * r, P * c)
            q1 = REG * (r + 1)
            if q1 > q0:
                out.append((r, q0, q1 - q0))
        return out

    last_c = [min(TK, RPB * (r + 1)) - 1 for r in range(NREG)]

    with ExitStack() as ctx:
        tc = ctx.enter_context(tile.TileContext(nc))
        const = ctx.enter_context(tc.tile_pool(name="const", bufs=1))
        rawq = ctx.enter_context(tc.tile_pool(name="rawq", bufs=1))
        vmixp = ctx.enter_context(tc.tile_pool(name="vmix", bufs=1))
        vraw = ctx.enter_context(tc.tile_pool(name="vraw", bufs=1))
        accp = ctx.enter_context(tc.tile_pool(name="acc", bufs=1))
        x2p = ctx.enter_context(tc.tile_pool(name="x2p", bufs=2))
        mixp = ctx.enter_context(tc.tile_pool(name="mix", bufs=2))
        scr = ctx.enter_context(tc.tile_pool(name="scr", bufs=2))
        normp = ctx.enter_context(tc.tile_pool(name="norm", bufs=2))
        pta = ctx.enter_context(tc.tile_pool(name="pta", bufs=PTA_BUFS))
        ptb = ctx.enter_context(tc.tile_pool(name="ptb", bufs=PTB_BUFS))

        pick = _Pick(nc)

        # DMAs are emitted in first-use order (the DMA device is serial in
        # the model): A tables -> head-0 slices -> v chunks -> later heads ->
        # B-only tables/slices. Raw q/k persistent tiles are filled by
        # per-head slice DMAs so head 0's data lands first.
        onesA = const.tile([P, P], F16)
        onesB = const.tile([P, P], F16)
        nc.sync.dma_start(out=onesA, in_=D["onesA"][:, :])
        nc.sync.dma_start(out=onesB, in_=D["onesB"][:, :])
        tabs = {}

        def load_tab(nm):
            rows = _in_specs(cfg)[nm][0]
            tl = const.tile([P, rows // P, T], F16, name=nm, tag=nm)
            tabs[nm] = tl
            nc.sync.dma_start(out=tl, in_=D[nm].rearrange("(c p) t -> p c t", p=P))

        wrs = {}

        def load_wrs():
            for nm in ("wr0", "wr2"):
                cols = _in_specs(cfg)[nm][1]
                tl = const.tile([P, cols], F16, name=nm, tag=nm)
                wrs[nm] = tl
                nc.sync.dma_start(out=tl, in_=D[nm][:, :])

        raw = {}
        for nm in ("qA1", "qB1s", "kA1", "kB1", "kB1s"):
            rows = _in_specs(cfg)[nm][0]
            raw[nm] = rawq.tile([P, rows // P, T], F16, name=nm, tag=nm)
        for nm in ("qA1s", "kA1s"):  # device-built sigma64 copies
            raw[nm] = rawq.tile([P, 4, T], F16, name=nm, tag=nm)

        def emit_sigma64(nm, h):
            s_, d_ = raw[nm[:3]], raw[nm]
            pick.copy16(d_[0:64, h, :], s_[64:128, h, :], 1024)
            pick.copy16(d_[64:128, h, :], s_[0:64, h, :], 1024)

        def load_raw_slice(nm, i):
            nc.sync.dma_start(out=raw[nm][:, i, :],
                              in_=D[nm][P * i:P * (i + 1), :])

        def load_x2(dram, pair, tag):
            tl = x2p.tile([P, 2, T], F16, tag=tag, name=tag)
            for i in range(2):  # split halves: part 0 usable earlier
                nc.sync.dma_start(
                    out=tl[:, i, :],
                    in_=dram[P * (2 * pair + i):P * (2 * pair + i + 1), :])
            return tl

        # head-0 loads, interleaved with the tables each mix op needs so the
        # first products can start ~4us in; wrs (v-mix weights) come last
        q2_0 = x2p.tile([P, 2, T], F16, tag="x2a", name="x2a")
        k2_0 = x2p.tile([P, 2, T], F16, tag="x2b", name="x2b")
        nc.sync.dma_start(out=q2_0[:, 0, :], in_=D["qA2"][0:P, :])
        load_tab("ca2")
        nc.sync.dma_start(out=q2_0[:, 1, :], in_=D["qA2"][P:2 * P, :])
        load_tab("sa2")
        nc.sync.dma_start(out=k2_0[:, 0, :], in_=D["kA2"][0:P, :])
        nc.sync.dma_start(out=k2_0[:, 1, :], in_=D["kA2"][P:2 * P, :])
        load_raw_slice("kA1", 0)
        load_raw_slice("qA1", 0)
        load_tab("ca1")
        load_tab("sa1")
        load_wrs()

        # v tiles; vA2 is DMA'd straight into vmixA and scaled in place.
        # Chunk DMAs are interleaved with head-1's loads further down so the
        # serial DMA queue tracks consumption order.
        vmixA = vmixp.tile([P, TK, 4, 256], F16)
        vmixB = vmixp.tile([P, TK, 4, 128], F16)
        vB1r = vraw.tile([P, TK, 256], F16)

        def load_v_chunk(c):
            nc.sync.dma_start(out=vmixA[:, c],
                              in_=D["vA2"][P * c:P * (c + 1), :])
            nc.sync.dma_start(out=vmixB[:, c],
                              in_=D["vA1"][P * c:P * (c + 1), :])

        # pt is softmax-scale-invariant, so w1/w3 live in the den "ones"
        # columns: vmixA = vA2 + (w0/w1)*vA1 pad; vmixB = vA1 + (w2/w3)*vB1.
        # vmixA's ratio-mult reads vmixB (raw vA1) BEFORE emit_vmixB's add
        # modifies its first 64 cols per kv -- emission order guarantees it.

        def emit_vmixA():
            for c in range(TK):
                vt1 = scr.tile([P, 4, 128], F16, tag="mt2")
                pick.tt16(vt1, vmixB[:, c, :], wrs["wr0"], mult, 512)
                pick.tt16(vmixA[:, c, :, 0:128], vmixA[:, c, :, 0:128], vt1,
                          add, 512)

        def emit_vmixB():
            for c in range(TK):
                vt2 = scr.tile([P, 4, 64], F16, tag="mu2")
                pick.tt16(vt2, vB1r[:, c, :], wrs["wr2"], mult, 256)
                pick.tt16(vmixB[:, c, :, 0:64], vmixB[:, c, :, 0:64], vt2,
                          add, 256)

        outacc = accp.tile([P, 8, T], F16)

        def mix_A_dc0(dst, x1, x1s, x2):
            """dst[:,0,:] = (x2_0*c2_0 + x2_1*s2_0) + (x1*c1 + x1s*s1)."""
            t = scr.tile([P, T], F16, tag="mt")
            u = scr.tile([P, T], F16, tag="mu")
            t2 = scr.tile([P, T], F16, tag="mt2")
            u2 = scr.tile([P, T], F16, tag="mu2")
            pick.tt16(t, x2[:, 0, :], tabs["ca2"][:, 0, :], mult, 1024)
            pick.tt16(u, x2[:, 1, :], tabs["sa2"][:, 0, :], mult, 1024)
            pick.tt16(t2, x1, tabs["ca1"][:, 0, :], mult, 1024)
            pick.tt16(u2, x1s, tabs["sa1"][:, 0, :], mult, 1024)
            pick.tt16(t, t, u, add, 1024)
            pick.tt16(t2, t2, u2, add, 1024)
            pick.tt16(dst[:, 0, :], t, t2, add, 1024)

        def mix_A_dc1(dst, x2):
            """dst[:,1,:] = x2_1*c2_1 + x2_0*s2_1."""
            t = scr.tile([P, T], F16, tag="mt")
            u = scr.tile([P, T], F16, tag="mu")
            pick.tt16(t, x2[:, 1, :], tabs["ca2"][:, 1, :], mult, 1024)
            pick.tt16(u, x2[:, 0, :], tabs["sa2"][:, 1, :], mult, 1024)
            pick.tt16(dst[:, 1, :], t, u, add, 1024)

        def mix_B_pair(dst, x2, x2s, x1, x1s):
            """dst [P,2,T]: B-heads (2j,2j+1) rope mix.
            x2/x2s [P,2,T] d128 nat/sigma64; x1/x1s [P,T] packed d64 pair."""
            for hh in range(2):
                t2 = scr.tile([P, T], F16, tag="mt")
                u2 = scr.tile([P, T], F16, tag="mu")
                pick.tt16(t2, x2[:, hh, :], tabs["cb2"][:, 0, :], mult, 1024)
                pick.tt16(u2, x2s[:, hh, :], tabs["sb2"][:, 0, :], mult, 1024)
                pick.tt16(dst[:, hh, :], t2, u2, add, 1024)
            # packed d64 part for both heads
            t = scr.tile([P, T], F16, tag="mt2")
            u = scr.tile([P, T], F16, tag="mu2")
            pick.tt16(t, x1, tabs["cb1"][:, 0, :], mult, 1024)
            pick.tt16(u, x1s, tabs["sb1"][:, 0, :], mult, 1024)
            pick.tt16(t, t, u, add, 1024)
            # head 2j: rows 0:64 aligned; head 2j+1: cross-base copy first
            pick.tt16(dst[0:64, 0, :], dst[0:64, 0, :], t[0:64, :], add, 1024)
            pick.copy16(u[0:64, :], t[64:128, :], 1024)
            pick.tt16(dst[0:64, 1, :], dst[0:64, 1, :], u[0:64, :], add, 1024)

        def attn_head(qmix_dcs, kmix_dcs, vmix_ap, blks, spool, ypool, dpool,
                      wide_sT, is_b, lag=1, mid_emit=None):
            ones = onesB if is_b else onesA
            """qmix_dcs/kmix_dcs: list of [P,T] APs (one per 128-d chunk).
            vmix_ap(c, dc) -> stationary [P,128]. blks: output block ids.
            Emits PE work software-pipelined: scores(c) ... y/den(c-lag)."""
            ndc = len(qmix_dcs)
            den = dpool.tile([P, T], F32, tag="den")
            yts = [ypool.tile([P, T], F32, tag=f"yt{i}", name=f"yt{i}")
                   for i in range(ndc)]
            pending = []  # deferred (c, parts) lists, flushed `lag` behind

            def emit_norm(r):
                # region r's den/y accumulation is complete: normalize and
                # (for B) add+store now, overlapping the remaining chunks.
                # rec stays in fp16's normal range: the host pre-scales v and
                # the ones columns by 1/16 (softmax is pt-scale-invariant).
                sl = slice(REG * r, REG * (r + 1))
                rec = normp.tile([P, REG], F16, tag="rec")
                with nc.allow_low_precision(
                        reason="softmax rec in fp16; host pre-scales 1/16 "
                               "so rec stays in fp16 normal range"):
                    pick.recip(rec, den[:, sl], REG)
                for dc in range(ndc):
                    blk = blks[dc]
                    y_sb = normp.tile([P, REG], F16, tag="ysb")
                    pick.copy_ps(y_sb, yts[dc][:, sl], REG, on_act=not is_b)
                    if not is_b:
                        pick.tt16(outacc[:, blk, sl], y_sb, rec, mult, 512)
                    else:
                        tmp = normp.tile([P, REG], F16, tag="btmp")
                        pick.tt16(tmp, y_sb, rec, mult, 512)
                        pick.tt16(outacc[:, blk, sl], outacc[:, blk, sl],
                                  tmp, add, 512)
                        nc.sync.dma_start(
                            out=outT[P * blk:P * (blk + 1), sl],
                            in_=outacc[:, blk, sl])

            def flush(pend):
                c, parts = pend
                for (q0, n, pt, off) in parts:
                    r = q0 // REG
                    # the diagonal 128 cols wait on the affine mask; split
                    # them off so the bulk matmuls only depend on the exp.
                    # (not at c==0: two start=True groups in one PSUM zero
                    # region are illegal)
                    segs = [(q0, n, off)]
                    if q0 == P * c and n > P and c > 0:
                        segs = [(q0 + P, n - P, off + P), (q0, P, off)]
                    for (s0, sn, so) in segs:
                        for dc in range(ndc):
                            nc.tensor.matmul(
                                yts[dc][:, s0:s0 + sn], vmix_ap(c, dc),
                                pt[:, so:so + sn],
                                start=(c == 0), stop=(c == last_c[r]))
                        nc.tensor.matmul(den[:, s0:s0 + sn], ones,
                                         pt[:, so:so + sn],
                                         start=(c == 0), stop=(c == last_c[r]))
                for r in range(NREG):
                    if last_c[r] == c:
                        emit_norm(r)

            for c in range(TK):
                if c == 3 and mid_emit is not None:
                    mid_emit()
                parts = []
                if wide_sT:
                    # one [P,T] sT + one exp for the whole chunk
                    sT = spool.tile([P, T], F32, tag="sTw")
                    pt = ptb.tile([P, T], F16, tag="ptw")
                    for (r, q0, n) in subchunks(c):
                        for dc in range(ndc):
                            nc.tensor.matmul(
                                sT[:, q0:q0 + n],
                                kmix_dcs[dc][:, P * c:P * (c + 1)],
                                qmix_dcs[dc][:, q0:q0 + n],
                                start=(dc == 0), stop=(dc == ndc - 1))
                    pick.act_exp(pt[:, P * c:], sT[:, P * c:], T - P * c)
                    parts = [(q0, n, pt, q0) for (r, q0, n) in subchunks(c)]
                    diag = (pt, P * c)
                else:
                    # [P,REG] sT + exp per subchunk
                    diag = None
                    for (r, q0, n) in subchunks(c):
                        sT = spool.tile([P, REG], F32, tag="sTn")
                        pt = pta.tile([P, REG], F16, tag="ptn")
                        for dc in range(ndc):
                            nc.tensor.matmul(
                                sT[:, 0:n],
                                kmix_dcs[dc][:, P * c:P * (c + 1)],
                                qmix_dcs[dc][:, q0:q0 + n],
                                start=(dc == 0), stop=(dc == ndc - 1))
                        pick.act_exp(pt[:, 0:n], sT[:, 0:n], n)
                        if q0 == P * c:
                            diag = (pt, 0)
                        parts.append((q0, n, pt, 0))
                # zero the masked (k>q) half of the causal diagonal block
                dpt, doff = diag
                pick.pool_fix(128).affine_select(
                    out=dpt[:, doff:doff + P], in_=dpt[:, doff:doff + P],
                    compare_op=mybir.AluOpType.is_ge, fill=0.0,
                    base=0, pattern=[[1, P]], channel_multiplier=-1)
                pending.append((c, parts))
                if len(pending) > lag:
                    flush(pending.pop(0))
            for pend in pending:
                flush(pend)

        def mix_A_head(h, q2=None, k2=None):
            # DMAs emitted here so the serial DMA queue runs in use order;
            # dc0 chains for q AND k first so the first score matmul (which
            # only needs dc0 of both) is unblocked as early as possible.
            if q2 is None:
                q2 = load_x2(D["qA2"], h, "x2a")
                load_raw_slice("qA1", h)
                k2 = load_x2(D["kA2"], h, "x2b")
                load_raw_slice("kA1", h)
            emit_sigma64("qA1s", h)
            emit_sigma64("kA1s", h)
            qmix = mixp.tile([P, 2, T], F16, tag="qmix")
            kmix = mixp.tile([P, 2, T], F16, tag="kmix")
            mix_A_dc0(qmix, raw["qA1"][:, h, :], raw["qA1s"][:, h, :], q2)
            mix_A_dc0(kmix, raw["kA1"][:, h, :], raw["kA1s"][:, h, :], k2)
            mix_A_dc1(qmix, q2)
            mix_A_dc1(kmix, k2)
            return qmix, kmix

        def mix_B_group(j):
            """kv j's kmix + the q pair for heads (2j, 2j+1)."""
            if j == 0:
                for nm in ("cb2", "sb2", "cb1", "sb1"):
                    load_tab(nm)
            if j % 2 == 0:
                load_raw_slice("kB1", j // 2)
                load_raw_slice("kB1s", j // 2)
            load_raw_slice("qB1s", j)
            kmix = mixp.tile([P, T], F16, tag="kmixB")
            t = scr.tile([P, T], F16, tag="mt2")
            u = scr.tile([P, T], F16, tag="mu2")
            pick.tt16(t, raw["kA1"][:, j, :], tabs["cb2"][:, 0, :], mult, 1024)
            pick.tt16(u, raw["kA1s"][:, j, :], tabs["sb2"][:, 0, :], mult, 1024)
            pick.tt16(kmix, t, u, add, 1024)
            # d64 part: computed once per kv pair (kB1 tile j//2), reused by
            # the odd kv via a cross-base copy
            jp, g = j // 2, j % 2
            if g == 0:
                t64 = scr.tile([P, T], F16, tag="t64")
                pick.tt16(t64, raw["kB1"][:, jp, :], tabs["cb1"][:, 0, :],
                          mult, 1024)
                pick.tt16(u, raw["kB1s"][:, jp, :], tabs["sb1"][:, 0, :],
                          mult, 1024)
                pick.tt16(t64, t64, u, add, 1024)
                st["t64"] = t64
                pick.tt16(kmix[0:64, :], kmix[0:64, :], t64[0:64, :], add, 1024)
            else:
                pick.copy16(u[0:64, :], st["t64"][64:128, :], 1024)
                pick.tt16(kmix[0:64, :], kmix[0:64, :], u[0:64, :], add, 1024)

            q2 = load_x2(D["qA2"], j, "x2a")
            q2s = load_x2(D["qB2s"], j, "x2b")
            qmixp = mixp.tile([P, 2, T], F16, tag="qmix")
            mix_B_pair(qmixp, q2, q2s,
                       raw["qA1"][:, j, :], raw["qB1s"][:, j, :])
            return qmixp, kmix

        # ============ config A (B group 0's mix overlaps A3's attn) ========
        st = {"amix": mix_A_head(0, q2_0, k2_0), "bmix": None}
        # interleave v-chunk DMAs with head 1's loads in the serial DMA queue
        load_v_chunk(0)
        load_v_chunk(1)
        q2_1 = load_x2(D["qA2"], 1, "x2a")
        load_raw_slice("qA1", 1)
        load_v_chunk(2)
        load_v_chunk(3)
        k2_1 = load_x2(D["kA2"], 1, "x2b")
        load_raw_slice("kA1", 1)
        for c in range(4, TK):
            load_v_chunk(c)
        emit_vmixA()

        with tc.tile_pool(name="spsA", bufs=2, space="PSUM") as spsA, \
             tc.tile_pool(name="ypsA", bufs=1, space="PSUM") as ypsA, \
             tc.tile_pool(name="dpsA", bufs=1, space="PSUM") as dpsA:
            for h in range(cfg.NA):
                qmix, kmix = st["amix"]

                def filler(h=h):
                    if h == 0:
                        st["amix"] = mix_A_head(1, q2_1, k2_1)
                        for c in range(TK):
                            nc.sync.dma_start(out=vB1r[:, c, :],
                                              in_=D["vB1"][P * c:P * (c + 1), :])
                    elif h == 1:
                        st["amix"] = mix_A_head(2)
                        emit_vmixB()
                    elif h == 2:
                        st["amix"] = mix_A_head(3)
                    else:
                        st["bmix"] = mix_B_group(0)

                def vap(c, dc, h=h):
                    return vmixA[:, c, h, 128 * dc:128 * (dc + 1)]

                attn_head([qmix[:, 0, :], qmix[:, 1, :]],
                          [kmix[:, 0, :], kmix[:, 1, :]],
                          vap, (2 * h, 2 * h + 1), spsA, ypsA, dpsA,
                          wide_sT=False, is_b=False, lag=LAG_A, mid_emit=filler)

        # ================= config B =================
        with tc.tile_pool(name="spsB", bufs=2, space="PSUM") as spsB, \
             tc.tile_pool(name="ypsB", bufs=1, space="PSUM") as ypsB, \
             tc.tile_pool(name="dpsB", bufs=1, space="PSUM") as dpsB:
            for j in range(4):  # kv head j serves B-heads (2j, 2j+1)
                qmixp, kmix = st["bmix"]

                def vapB(c, dc, j=j):
                    return vmixB[:, c, j, :]

                for hh in range(2):
                    b = 2 * j + hh
                    filler = None
                    if hh == 1 and j + 1 < 4:
                        def filler(j=j):
                            st["bmix"] = mix_B_group(j + 1)
                    attn_head([qmixp[:, hh, :]], [kmix], vapB, (b,),
                              spsB, ypsB, dpsB, wide_sT=True, is_b=True,
                              lag=LAG_B, mid_emit=filler)

    nc.compile()
    return nc


# ---------------------------------------------------------------------------
# Host side
# ---------------------------------------------------------------------------

def _rope_tab(pos, d, f):
    """Rope tables [d, T]: (f*cos, signed f*sin); sign folded so that
    out[j] = x[j]*c[j] + x[sigma(j)]*s[j] with sigma the half-swap."""
    inv = 1.0 / (10000.0 ** (np.arange(0, d, 2, dtype=np.float32) / d))
    ang = inv[:, None] * pos[None, :].astype(np.float32)      # [d/2, T]
    ang = np.concatenate([ang, ang], 0)                        # [d, T]
    c = (f * np.cos(ang)).astype(np.float32)
    s = (f * np.sin(ang)).astype(np.float32)
    s[: d // 2] *= -1.0
    return c, s


def _sigma(x, half, group):
    """Swap half-blocks of `half` rows within each `group`-row group."""
    r = x.reshape(-1, 2, half, x.shape[-1])
    assert group == 2 * half
    return np.ascontiguousarray(r[:, ::-1].reshape(x.shape))


def make_core_inputs(q, k, v, pos, weights, s, cfg: KCfg = FULL):
    """q,k,v: [T, 2048] fp32 for one batch; returns per-core input dict."""
    f16 = lambda a: np.ascontiguousarray(a, dtype=np.float16)
    qT1 = np.ascontiguousarray(q[:, 512 * s:512 * s + 512].T)
    qT2 = np.ascontiguousarray(q[:, 1024 * s:1024 * s + 1024].T)
    kT1 = np.ascontiguousarray(k[:, 512 * s:512 * s + 512].T)
    kT2 = np.ascontiguousarray(k[:, 1024 * s:1024 * s + 1024].T)
    kB1 = np.ascontiguousarray(k[:, 256 * s:256 * s + 256].T)
    arrs = {
        "qA1": f16(qT1),
        "qB1s": f16(_sigma(qT1, 32, 64)),
        "qA2": f16(qT2), "qB2s": f16(_sigma(qT2, 64, 128)),
        "kA1": f16(kT1),
        "kA2": f16(kT2),
        "kB1": f16(kB1), "kB1s": f16(_sigma(kB1, 32, 64)),
        "vA1": f16(v[:, 512 * s:512 * s + 512] / 16.0),
        "vA2": f16(v[:, 1024 * s:1024 * s + 1024] / 16.0),
        "vB1": f16(v[:, 256 * s:256 * s + 256] / 16.0),
    }
    fA = math.sqrt(1.0 / 16.0)
    fB = math.sqrt(1.0 / math.sqrt(128.0))
    ca1, sa1 = _rope_tab(pos, 128, fA * float(weights[0]))
    ca2, sa2 = _rope_tab(pos, 256, fA * float(weights[1]))
    cb1h, sb1h = _rope_tab(pos, 64, fB * float(weights[2]))
    cb2, sb2 = _rope_tab(pos, 128, fB * float(weights[3]))
    arrs.update({
        "ca1": f16(ca1), "sa1": f16(sa1),
        "ca2": f16(ca2), "sa2": f16(sa2),
        "cb1": f16(np.vstack([cb1h, cb1h])), "sb1": f16(np.vstack([sb1h, sb1h])),
        "cb2": f16(cb2), "sb2": f16(sb2),
        # pt is scale-invariant under softmax: fold w1 (w3) into the den
        # "ones" columns and keep only the w0/w1 (w2/w3) ratio on the v1 add
        "wr0": f16(np.full((P, 512), float(weights[0] / weights[1]))),
        "wr2": f16(np.full((P, 256), float(weights[2] / weights[3]))),
        "onesA": f16(np.full((P, 128), float(1.0 / (16.0 * weights[1])))),
        "onesB": f16(np.full((P, 128), float(1.0 / (16.0 * weights[3])))),
    })
    return arrs


_PROGRAM_CACHE = {}
TRACE = False
LAST_RESULT = None
NEG = -1e9


def kernel(q_m, k_m, v_m, weights, attention_mask, position_ids):
    global LAST_RESULT
    from concourse.bass_utils import run_bass_kernel_spmd

    cfg = FULL
    q_m = np.asarray(q_m, np.float32)
    k_m = np.asarray(k_m, np.float32)
    v_m = np.asarray(v_m, np.float32)
    weights = np.asarray(weights, np.float32)
    attention_mask = np.asarray(attention_mask, np.float32)
    position_ids = np.asarray(position_ids)
    B, T, H = q_m.shape

    # the device program hardcodes the causal structure; verify it holds
    causal = np.where(np.tril(np.ones((T, T), bool)), 0.0, NEG).astype(np.float32)
    for b in range(B):
        assert np.array_equal(attention_mask[b, 0], causal), "non-causal mask"

    if "nc" not in _PROGRAM_CACHE:
        _PROGRAM_CACHE["nc"] = build_program(cfg)
    nc = _PROGRAM_CACHE["nc"]

    in_maps = []
    for b in range(B):
        for s in range(2):
            in_maps.append(make_core_inputs(
                q_m[b], k_m[b], v_m[b], position_ids[b], weights, s, cfg))
    res = run_bass_kernel_spmd(nc, in_maps, list(range(8)), trace=TRACE)
    LAST_RESULT = res
    out = np.zeros((B, T, H), np.float32)
    for b in range(B):
        for s in range(2):
            out[b, :, 1024 * s:1024 * s + 1024] = \
                res.results[2 * b + s]["outT"].astype(np.float32).T
    return out


# revision 69
# speedup vs baseline: 1.8211x; 1.0046x over previous
"""Trainium2 Bass kernel for nn_MixedAttnHeadEmbed (mixed-head-config attention).

Math (per batch b):
  Two attention configs share q_m/k_m/v_m [B,T,2048]:
    A: h=8  heads, d_max=256, mixing e in {1024,2048} -> d in {128,256}, weights w0,w1
    B: h=16 heads, d_max=128, mixing e in {1024,2048} -> d in {64,128},  weights w2,w3
  Each config: per-head q/k slices are RoPE'd, weight-summed (padded to d_max),
  GQA (8 kv heads), causal softmax attention; outputs of both configs sum.

Sharding: 8 cores = 4 batches x 2 shards. Shard s owns A-heads [4s,4s+4) and
B-heads [8s,8s+8) -> both write output columns [1024s, 1024s+1024) which are
summed on device; per-core output is the transposed block outT [1024, T] fp16.

Device design (driven by the CoreSim cost model):
  * All on-device data is fp16 (PSUM accumulation stays fp32): DVE gets the
    2x fast mode for 2-byte dtypes, the PE runs 1 col/cycle at any moving
    width (f32r pays 4x under 256 cols), and DMA bytes halve. fp16's 5e-4
    epsilon keeps the end-to-end error ~1e-3, far under the 2e-2 gate.
  * RoPE rotations are eliminated on device: the host uploads sigma-permuted
    row copies of each q/k slice (rows swapped within each rotation group),
    so rope+mix is a chain of partition-aligned tensor_tensor ops against
    sign-folded sin/cos tables (weights and 1/sqrt(d) folded in on host).
  * Scores are computed transposed (sT[k,q], k on partitions) so softmax'd
    weights feed the y^T matmul with no transposes; softmax is max-free
    (scores provably < 2) with the denominator from an all-ones matmul.
  * Causal diag-block masking zeroes pt after the exp via a Pool
    affine_select instead of adding a mask into PSUM on DVE.
  * The PE stream is software-pipelined: scores for k-chunk c+1 are issued
    before y/den for chunk c, so the PE does not sit behind each exp.
"""

import math
from contextlib import ExitStack
from dataclasses import dataclass

import numpy as np

import concourse.bass as bass
import concourse.mybir as mybir
import concourse.tile as tile
from concourse import bacc

F32 = mybir.dt.float32
PTA_BUFS, PTB_BUFS, LAG_A, LAG_B = 10, 7, 4, 5
F16 = mybir.dt.float16
P = 128


@dataclass(frozen=True)
class KCfg:
    T: int = 1024       # sequence length
    NA: int = 4         # config-A heads per core (d_max=256)
    NB: int = 8         # config-B heads per core (d_max=128)
    REG: int = 512      # psum region width

    @property
    def TK(self):
        return self.T // P

    @property
    def NREG(self):
        return self.T // self.REG


FULL = KCfg()


def _in_specs(cfg: KCfg):
    T = cfg.T
    return {
        # q/k transposed slices (rows = head dims, fp16). *s = sigma-permuted
        # rows (rotation pairing partner), so rope is all aligned TT ops.
        "qA1": (512, T), "qB1s": (512, T),
        "qA2": (1024, T), "qB2s": (1024, T),
        "kA1": (512, T),
        "kA2": (1024, T),
        "kB1": (256, T), "kB1s": (256, T),
        # v slices, natural [T, d] layout
        "vA1": (T, 512), "vA2": (T, 1024), "vB1": (T, 256),
        # rope tables [d, T], weights+scale folded, sin sign-folded
        "ca1": (128, T), "sa1": (128, T),
        "ca2": (256, T), "sa2": (256, T),
        "cb1": (128, T), "sb1": (128, T),
        "cb2": (128, T), "sb2": (128, T),
        # v mixing: ratio rows (w0/w1, w2/w3) and 1/w den-ones columns
        "wr0": (P, 512), "wr2": (P, 256),
        "onesA": (P, 128), "onesB": (P, 128),
    }


class _Pick:
    """Static DVE-vs-Pool load balancer with cost-model-accurate weights.

    DVE: n*0.5208ns fp16 TT (2x mode), n*0.26 fp16 copy (4x), n*1.0417
    for psum/fp32 ops (+60/+125ns access). Pool: n*0.8333 flat. ACT is
    reserved for the exps (it is the 2nd-busiest engine)."""

    def __init__(self, nc):
        self.nc = nc
        self.load = {"dve": 0.0, "pool": 0.0}

    def _eng(self, cd, cp):
        if self.load["dve"] + cd <= self.load["pool"] + cp:
            self.load["dve"] += cd
            return self.nc.vector
        self.load["pool"] += cp
        return self.nc.gpsimd

    def tt16(self, out, in0, in1, op, n):
        e = self._eng(n * 0.5208 + 60, n * 0.8333 + 25)
        e.tensor_tensor(out, in0, in1, op)

    def recip(self, out, in_, n):
        """PSUM->SBUF reciprocal: DVE only (Pool has no PSUM port)."""
        self.load["dve"] += n * 1.0417 + 125
        self.nc.vector.reciprocal(out, in_)

    def act_exp(self, out, in_, n):
        self.load["act"] = self.load.get("act", 0.0) + n * 0.8333 + 185
        self.nc.scalar.activation(out, in_, mybir.ActivationFunctionType.Exp)

    def copy_ps(self, dst, src, n, on_act):
        """PSUM->SBUF copy. Phase-aware placement: ACT idles during config A
        (PE-bound) but saturates during config B, where DVE/Pool idle --
        so A-head copies go to ACT ('copy' shares the exp table, no reload)
        and B-head copies go to DVE."""
        if on_act:
            self.load["act"] = self.load.get("act", 0.0) + n * 0.8333 + 185
            self.nc.scalar.copy(dst, src)
        else:
            self.load["dve"] += n * 1.0417 + 125
            self.nc.vector.tensor_copy(dst, src)

    def copy16(self, dst, src, n):
        e = self._eng(n * 0.26 + 60, n * 0.8333 + 25)
        e.tensor_copy(dst, src)

    def pool_fix(self, n):
        self.load["pool"] += n * 0.8333 + 25
        return self.nc.gpsimd


def build_program(cfg: KCfg = FULL):
    nc = bacc.Bacc("TRN2", target_bir_lowering=False)
    T, TK, REG, NREG = cfg.T, cfg.TK, cfg.REG, cfg.NREG
    RPB = REG // P

    D = {}
    for name, shape in _in_specs(cfg).items():
        D[name] = nc.declare_dram_parameter(name, list(shape), F16, isOutput=False)
    outT = nc.declare_dram_parameter("outT", [1024, T], F16, isOutput=True)

    mult, add = mybir.AluOpType.mult, mybir.AluOpType.add

    def subchunks(c):
        out = []
        for r in range(NREG):
            q0 = max(REG * r, P * c)
            q1 = REG * (r + 1)
            if q1 > q0:
                out.append((r, q0, q1 - q0))
        return out

    last_c = [min(TK, RPB * (r + 1)) - 1 for r in range(NREG)]

    with ExitStack() as ctx:
        tc = ctx.enter_context(tile.TileContext(nc))
        const = ctx.enter_context(tc.tile_pool(name="const", bufs=1))
        rawq = ctx.enter_context(tc.tile_pool(name="rawq", bufs=1))
        vmixp = ctx.enter_context(tc.tile_pool(name="vmix", bufs=1))
        vraw = ctx.enter_context(tc.tile_pool(name="vraw", bufs=1))
        accp = ctx.enter_context(tc.tile_pool(name="acc", bufs=1))
        x2p = ctx.enter_context(tc.tile_pool(name="x2p", bufs=2))
        mixp = ctx.enter_context(tc.tile_pool(name="mix", bufs=2))
        scr = ctx.enter_context(tc.tile_pool(name="scr", bufs=2))
        normp = ctx.enter_context(tc.tile_pool(name="norm", bufs=2))
        pta = ctx.enter_context(tc.tile_pool(name="pta", bufs=PTA_BUFS))
        ptb = ctx.enter_context(tc.tile_pool(name="ptb", bufs=PTB_BUFS))

        pick = _Pick(nc)

        # DMAs are emitted in first-use order (the DMA device is serial in
        # the model): A tables -> head-0 slices -> v chunks -> later heads ->
        # B-only tables/slices. Raw q/k persistent tiles are filled by
        # per-head slice DMAs so head 0's data lands first.
        onesA = const.tile([P, P], F16)
        onesB = const.tile([P, P], F16)
        nc.sync.dma_start(out=onesA, in_=D["onesA"][:, :])
        nc.sync.dma_start(out=onesB, in_=D["onesB"][:, :])
        tabs = {}

        def load_tab(nm):
            rows = _in_specs(cfg)[nm][0]
            tl = const.tile([P, rows // P, T], F16, name=nm, tag=nm)
            tabs[nm] = tl
            nc.sync.dma_start(out=tl, in_=D[nm].rearrange("(c p) t -> p c t", p=P))

        wrs = {}

        def load_wrs():
            for nm in ("wr0", "wr2"):
                cols = _in_specs(cfg)[nm][1]
                tl = const.tile([P, cols], F16, name=nm, tag=nm)
                wrs[nm] = tl
                nc.sync.dma_start(out=tl, in_=D[nm][:, :])

        raw = {}
        for nm in ("qA1", "qB1s", "kA1", "kB1", "kB1s"):
            rows = _in_specs(cfg)[nm][0]
            raw[nm] = rawq.tile([P, rows // P, T], F16, name=nm, tag=nm)
        for nm in ("qA1s", "kA1s"):  # device-built sigma64 copies
            raw[nm] = rawq.tile([P, 4, T], F16, name=nm, tag=nm)

        def emit_sigma64(nm, h):
            s_, d_ = raw[nm[:3]], raw[nm]
            pick.copy16(d_[0:64, h, :], s_[64:128, h, :], 1024)
            pick.copy16(d_[64:128, h, :], s_[0:64, h, :], 1024)

        def load_raw_slice(nm, i):
            nc.sync.dma_start(out=raw[nm][:, i, :],
                              in_=D[nm][P * i:P * (i + 1), :])

        def load_x2(dram, pair, tag):
            tl = x2p.tile([P, 2, T], F16, tag=tag, name=tag)
            for i in range(2):  # split halves: part 0 usable earlier
                nc.sync.dma_start(
                    out=tl[:, i, :],
                    in_=dram[P * (2 * pair + i):P * (2 * pair + i + 1), :])
            return tl

        # head-0 loads, interleaved with the tables each mix op needs so the
        # first products can start ~4us in; wrs (v-mix weights) come last
        q2_0 = x2p.tile([P, 2, T], F16, tag="x2a", name="x2a")
        k2_0 = x2p.tile([P, 2, T], F16, tag="x2b", name="x2b")
        nc.sync.dma_start(out=q2_0[:, 0, :], in_=D["qA2"][0:P, :])
        load_tab("ca2")
        nc.sync.dma_start(out=q2_0[:, 1, :], in_=D["qA2"][P:2 * P, :])
        load_tab("sa2")
        nc.sync.dma_start(out=k2_0[:, 0, :], in_=D["kA2"][0:P, :])
        nc.sync.dma_start(out=k2_0[:, 1, :], in_=D["kA2"][P:2 * P, :])
        load_raw_slice("kA1", 0)
        load_raw_slice("qA1", 0)
        load_tab("ca1")
        load_tab("sa1")
        load_wrs()

        # v tiles; vA2 is DMA'd straight into vmixA and scaled in place.
        # Chunk DMAs are interleaved with head-1's loads further down so the
        # serial DMA queue tracks consumption order.
        vmixA = vmixp.tile([P, TK, 4, 256], F16)
        vmixB = vmixp.tile([P, TK, 4, 128], F16)
        vB1r = vraw.tile([P, TK, 256], F16)

        def load_v_chunk(c):
            nc.sync.dma_start(out=vmixA[:, c],
                              in_=D["vA2"][P * c:P * (c + 1), :])
            nc.sync.dma_start(out=vmixB[:, c],
                              in_=D["vA1"][P * c:P * (c + 1), :])

        # pt is softmax-scale-invariant, so w1/w3 live in the den "ones"
        # columns: vmixA = vA2 + (w0/w1)*vA1 pad; vmixB = vA1 + (w2/w3)*vB1.
        # vmixA's ratio-mult reads vmixB (raw vA1) BEFORE emit_vmixB's add
        # modifies its first 64 cols per kv -- emission order guarantees it.

        def emit_vmixA():
            for c in range(TK):
                vt1 = scr.tile([P, 4, 128], F16, tag="mt2")
                pick.tt16(vt1, vmixB[:, c, :], wrs["wr0"], mult, 512)
                pick.tt16(vmixA[:, c, :, 0:128], vmixA[:, c, :, 0:128], vt1,
                          add, 512)

        def emit_vmixB():
            for c in range(TK):
                vt2 = scr.tile([P, 4, 64], F16, tag="mu2")
                pick.tt16(vt2, vB1r[:, c, :], wrs["wr2"], mult, 256)
                pick.tt16(vmixB[:, c, :, 0:64], vmixB[:, c, :, 0:64], vt2,
                          add, 256)

        outacc = accp.tile([P, 8, T], F16)

        def mix_A_dc0(dst, x1, x1s, x2):
            """dst[:,0,:] = (x2_0*c2_0 + x2_1*s2_0) + (x1*c1 + x1s*s1)."""
            t = scr.tile([P, T], F16, tag="mt")
            u = scr.tile([P, T], F16, tag="mu")
            t2 = scr.tile([P, T], F16, tag="mt2")
            u2 = scr.tile([P, T], F16, tag="mu2")
            pick.tt16(t, x2[:, 0, :], tabs["ca2"][:, 0, :], mult, 1024)
            pick.tt16(u, x2[:, 1, :], tabs["sa2"][:, 0, :], mult, 1024)
            pick.tt16(t2, x1, tabs["ca1"][:, 0, :], mult, 1024)
            pick.tt16(u2, x1s, tabs["sa1"][:, 0, :], mult, 1024)
            pick.tt16(t, t, u, add, 1024)
            pick.tt16(t2, t2, u2, add, 1024)
            pick.tt16(dst[:, 0, :], t, t2, add, 1024)

        def mix_A_dc1(dst, x2):
            """dst[:,1,:] = x2_1*c2_1 + x2_0*s2_1."""
            t = scr.tile([P, T], F16, tag="mt")
            u = scr.tile([P, T], F16, tag="mu")
            pick.tt16(t, x2[:, 1, :], tabs["ca2"][:, 1, :], mult, 1024)
            pick.tt16(u, x2[:, 0, :], tabs["sa2"][:, 1, :], mult, 1024)
            pick.tt16(dst[:, 1, :], t, u, add, 1024)

        def mix_B_pair(dst, x2, x2s, x1, x1s):
            """dst [P,2,T]: B-heads (2j,2j+1) rope mix.
            x2/x2s [P,2,T] d128 nat/sigma64; x1/x1s [P,T] packed d64 pair."""
            for hh in range(2):
                t2 = scr.tile([P, T], F16, tag="mt")
                u2 = scr.tile([P, T], F16, tag="mu")
                pick.tt16(t2, x2[:, hh, :], tabs["cb2"][:, 0, :], mult, 1024)
                pick.tt16(u2, x2s[:, hh, :], tabs["sb2"][:, 0, :], mult, 1024)
                pick.tt16(dst[:, hh, :], t2, u2, add, 1024)
            # packed d64 part for both heads
            t = scr.tile([P, T], F16, tag="mt2")
            u = scr.tile([P, T], F16, tag="mu2")
            pick.tt16(t, x1, tabs["cb1"][:, 0, :], mult, 1024)
            pick.tt16(u, x1s, tabs["sb1"][:, 0, :], mult, 1024)
            pick.tt16(t, t, u, add, 1024)
            # head 2j: rows 0:64 aligned; head 2j+1: cross-base copy first
            pick.tt16(dst[0:64, 0, :], dst[0:64, 0, :], t[0:64, :], add, 1024)
            pick.copy16(u[0:64, :], t[64:128, :], 1024)
            pick.tt16(dst[0:64, 1, :], dst[0:64, 1, :], u[0:64, :], add, 1024)

        def attn_head(qmix_dcs, kmix_dcs, vmix_ap, blks, spool, ypool, dpool,
                      wide_sT, is_b, lag=1, mid_emit=None):
            ones = onesB if is_b else onesA
            """qmix_dcs/kmix_dcs: list of [P,T] APs (one per 128-d chunk).
            vmix_ap(c, dc) -> stationary [P,128]. blks: output block ids.
            Emits PE work software-pipelined: scores(c) ... y/den(c-lag)."""
            ndc = len(qmix_dcs)
            den = dpool.tile([P, T], F32, tag="den")
            yts = [ypool.tile([P, T], F32, tag=f"yt{i}", name=f"yt{i}")
                   for i in range(ndc)]
            pending = []  # deferred (c, parts) lists, flushed `lag` behind

            def emit_norm(r):
                # region r's den/y accumulation is complete: normalize and
                # (for B) add+store now, overlapping the remaining chunks.
                # rec stays in fp16's normal range: the host pre-scales v and
                # the ones columns by 1/16 (softmax is pt-scale-invariant).
                sl = slice(REG * r, REG * (r + 1))
                rec = normp.tile([P, REG], F16, tag="rec")
                with nc.allow_low_precision(
                        reason="softmax rec in fp16; host pre-scales 1/16 "
                               "so rec stays in fp16 normal range"):
                    pick.recip(rec, den[:, sl], REG)
                for dc in range(ndc):
                    blk = blks[dc]
                    y_sb = normp.tile([P, REG], F16, tag="ysb")
                    pick.copy_ps(y_sb, yts[dc][:, sl], REG, on_act=not is_b)
                    if not is_b:
                        pick.tt16(outacc[:, blk, sl], y_sb, rec, mult, 512)
                    else:
                        tmp = normp.tile([P, REG], F16, tag="btmp")
                        pick.tt16(tmp, y_sb, rec, mult, 512)
                        pick.tt16(outacc[:, blk, sl], outacc[:, blk, sl],
                                  tmp, add, 512)
                        nc.sync.dma_start(
                            out=outT[P * blk:P * (blk + 1), sl],
                            in_=outacc[:, blk, sl])

            def flush(pend):
                c, parts = pend
                for (q0, n, pt, off) in parts:
                    r = q0 // REG
                    # the diagonal 128 cols wait on the affine mask; split
                    # them off so the bulk matmuls only depend on the exp.
                    # (not at c==0: two start=True groups in one PSUM zero
                    # region are illegal)
                    segs = [(q0, n, off)]
                    if q0 == P * c and n > P and c > 0:
                        segs = [(q0 + P, n - P, off + P), (q0, P, off)]
                    for (s0, sn, so) in segs:
                        for dc in range(ndc):
                            nc.tensor.matmul(
                                yts[dc][:, s0:s0 + sn], vmix_ap(c, dc),
                                pt[:, so:so + sn],
                                start=(c == 0), stop=(c == last_c[r]))
                        nc.tensor.matmul(den[:, s0:s0 + sn], ones,
                                         pt[:, so:so + sn],
                                         start=(c == 0), stop=(c == last_c[r]))
                for r in range(NREG):
                    if last_c[r] == c:
                        emit_norm(r)

            for c in range(TK):
                if c == 3 and mid_emit is not None:
                    mid_emit()
                parts = []
                if wide_sT:
                    # one [P,T] sT + one exp for the whole chunk
                    sT = spool.tile([P, T], F32, tag="sTw")
                    pt = ptb.tile([P, T], F16, tag="ptw")
                    for (r, q0, n) in subchunks(c):
                        for dc in range(ndc):
                            nc.tensor.matmul(
                                sT[:, q0:q0 + n],
                                kmix_dcs[dc][:, P * c:P * (c + 1)],
                                qmix_dcs[dc][:, q0:q0 + n],
                                start=(dc == 0), stop=(dc == ndc - 1))
                    pick.act_exp(pt[:, P * c:], sT[:, P * c:], T - P * c)
                    parts = [(q0, n, pt, q0) for (r, q0, n) in subchunks(c)]
                    diag = (pt, P * c)
                else:
                    # [P,REG] sT + exp per subchunk
                    diag = None
                    for (r, q0, n) in subchunks(c):
                        sT = spool.tile([P, REG], F32, tag="sTn")
                        pt = pta.tile([P, REG], F16, tag="ptn")
                        for dc in range(ndc):
                            nc.tensor.matmul(
                                sT[:, 0:n],
                                kmix_dcs[dc][:, P * c:P * (c + 1)],
                                qmix_dcs[dc][:, q0:q0 + n],
                                start=(dc == 0), stop=(dc == ndc - 1))
                        pick.act_exp(pt[:, 0:n], sT[:, 0:n], n)
                        if q0 == P * c:
                            diag = (pt, 0)
                        parts.append((q0, n, pt, 0))
                # zero the masked (k>q) half of the causal diagonal block
                dpt, doff = diag
                pick.pool_fix(128).affine_select(
                    out=dpt[:, doff:doff + P], in_=dpt[:, doff:doff + P],
                    compare_op=mybir.AluOpType.is_ge, fill=0.0,
                    base=0, pattern=[[1, P]], channel_multiplier=-1)
                pending.append((c, parts))
                if len(pending) > lag:
                    flush(pending.pop(0))
            for pend in pending:
                flush(pend)

        def mix_A_head(h, q2=None, k2=None):
            # DMAs emitted here so the serial DMA queue runs in use order;
            # dc0 chains for q AND k first so the first score matmul (which
            # only needs dc0 of both) is unblocked as early as possible.
            if q2 is None:
                q2 = load_x2(D["qA2"], h, "x2a")
                load_raw_slice("qA1", h)
                k2 = load_x2(D["kA2"], h, "x2b")
                load_raw_slice("kA1", h)
            emit_sigma64("qA1s", h)
            emit_sigma64("kA1s", h)
            qmix = mixp.tile([P, 2, T], F16, tag="qmix")
            kmix = mixp.tile([P, 2, T], F16, tag="kmix")
            mix_A_dc0(qmix, raw["qA1"][:, h, :], raw["qA1s"][:, h, :], q2)
            mix_A_dc0(kmix, raw["kA1"][:, h, :], raw["kA1s"][:, h, :], k2)
            mix_A_dc1(qmix, q2)
            mix_A_dc1(kmix, k2)
            return qmix, kmix

        def mix_B_group(j):
            """kv j's kmix + the q pair for heads (2j, 2j+1)."""
            if j == 0:
                for nm in ("cb2", "sb2", "cb1", "sb1"):
                    load_tab(nm)
            if j % 2 == 0:
                load_raw_slice("kB1", j // 2)
                load_raw_slice("kB1s", j // 2)
            load_raw_slice("qB1s", j)
            kmix = mixp.tile([P, T], F16, tag="kmixB")
            t = scr.tile([P, T], F16, tag="mt2")
            u = scr.tile([P, T], F16, tag="mu2")
            pick.tt16(t, raw["kA1"][:, j, :], tabs["cb2"][:, 0, :], mult, 1024)
            pick.tt16(u, raw["kA1s"][:, j, :], tabs["sb2"][:, 0, :], mult, 1024)
            pick.tt16(kmix, t, u, add, 1024)
            # d64 part: computed once per kv pair (kB1 tile j//2), reused by
            # the odd kv via a cross-base copy
            jp, g = j // 2, j % 2
            if g == 0:
                t64 = scr.tile([P, T], F16, tag="t64")
                pick.tt16(t64, raw["kB1"][:, jp, :], tabs["cb1"][:, 0, :],
                          mult, 1024)
                pick.tt16(u, raw["kB1s"][:, jp, :], tabs["sb1"][:, 0, :],
                          mult, 1024)
                pick.tt16(t64, t64, u, add, 1024)
                st["t64"] = t64
                pick.tt16(kmix[0:64, :], kmix[0:64, :], t64[0:64, :], add, 1024)
            else:
                pick.copy16(u[0:64, :], st["t64"][64:128, :], 1024)
                pick.tt16(kmix[0:64, :], kmix[0:64, :], u[0:64, :], add, 1024)

            q2 = load_x2(D["qA2"], j, "x2a")
            q2s = load_x2(D["qB2s"], j, "x2b")
            qmixp = mixp.tile([P, 2, T], F16, tag="qmix")
            mix_B_pair(qmixp, q2, q2s,
                       raw["qA1"][:, j, :], raw["qB1s"][:, j, :])
            return qmixp, kmix

        # ============ config A (B group 0's mix overlaps A3's attn) ========
        st = {"amix": mix_A_head(0, q2_0, k2_0), "bmix": None}
        # interleave v-chunk DMAs with head 1's loads in the serial DMA queue
        load_v_chunk(0)
        load_v_chunk(1)
        q2_1 = load_x2(D["qA2"], 1, "x2a")
        load_raw_slice("qA1", 1)
        load_v_chunk(2)
        load_v_chunk(3)
        k2_1 = load_x2(D["kA2"], 1, "x2b")
        load_raw_slice("kA1", 1)
        for c in range(4, TK):
            load_v_chunk(c)
        emit_vmixA()

        with tc.tile_pool(name="spsA", bufs=2, space="PSUM") as spsA, \
             tc.tile_pool(name="ypsA", bufs=1, space="PSUM") as ypsA, \
             tc.tile_pool(name="dpsA", bufs=1, space="PSUM") as dpsA:
            for h in range(cfg.NA):
                qmix, kmix = st["amix"]

                def filler(h=h):
                    # vB1/vmixB wait until h==2 so heads 2-3's DMA bundles
                    # are not queued behind them (vmixB is first used by B0)
                    if h == 0:
                        st["amix"] = mix_A_head(1, q2_1, k2_1)
                    elif h == 1:
                        st["amix"] = mix_A_head(2)
                    elif h == 2:
                        st["amix"] = mix_A_head(3)
                        for c in range(TK):
                            nc.sync.dma_start(out=vB1r[:, c, :],
                                              in_=D["vB1"][P * c:P * (c + 1), :])
                        emit_vmixB()
                    else:
                        st["bmix"] = mix_B_group(0)

                def vap(c, dc, h=h):
                    return vmixA[:, c, h, 128 * dc:128 * (dc + 1)]

                attn_head([qmix[:, 0, :], qmix[:, 1, :]],
                          [kmix[:, 0, :], kmix[:, 1, :]],
                          vap, (2 * h, 2 * h + 1), spsA, ypsA, dpsA,
                          wide_sT=False, is_b=False, lag=LAG_A, mid_emit=filler)

        # ================= config B =================
        with tc.tile_pool(name="spsB", bufs=2, space="PSUM") as spsB, \
             tc.tile_pool(name="ypsB", bufs=1, space="PSUM") as ypsB, \
             tc.tile_pool(name="dpsB", bufs=1, space="PSUM") as dpsB:
            for j in range(4):  # kv head j serves B-heads (2j, 2j+1)
                qmixp, kmix = st["bmix"]

                def vapB(c, dc, j=j):
                    return vmixB[:, c, j, :]

                for hh in range(2):
                    b = 2 * j + hh
                    filler = None
                    if hh == 1 and j + 1 < 4:
                        def filler(j=j):
                            st["bmix"] = mix_B_group(j + 1)
                    attn_head([qmixp[:, hh, :]], [kmix], vapB, (b,),
                              spsB, ypsB, dpsB, wide_sT=True, is_b=True,
                              lag=LAG_B, mid_emit=filler)

    nc.compile()
    return nc


# ---------------------------------------------------------------------------
# Host side
# ---------------------------------------------------------------------------

def _rope_tab(pos, d, f):
    """Rope tables [d, T]: (f*cos, signed f*sin); sign folded so that
    out[j] = x[j]*c[j] + x[sigma(j)]*s[j] with sigma the half-swap."""
    inv = 1.0 / (10000.0 ** (np.arange(0, d, 2, dtype=np.float32) / d))
    ang = inv[:, None] * pos[None, :].astype(np.float32)      # [d/2, T]
    ang = np.concatenate([ang, ang], 0)                        # [d, T]
    c = (f * np.cos(ang)).astype(np.float32)
    s = (f * np.sin(ang)).astype(np.float32)
    s[: d // 2] *= -1.0
    return c, s


def _sigma(x, half, group):
    """Swap half-blocks of `half` rows within each `group`-row group."""
    r = x.reshape(-1, 2, half, x.shape[-1])
    assert group == 2 * half
    return np.ascontiguousarray(r[:, ::-1].reshape(x.shape))


def make_core_inputs(q, k, v, pos, weights, s, cfg: KCfg = FULL):
    """q,k,v: [T, 2048] fp32 for one batch; returns per-core input dict."""
    f16 = lambda a: np.ascontiguousarray(a, dtype=np.float16)
    qT1 = np.ascontiguousarray(q[:, 512 * s:512 * s + 512].T)
    qT2 = np.ascontiguousarray(q[:, 1024 * s:1024 * s + 1024].T)
    kT1 = np.ascontiguousarray(k[:, 512 * s:512 * s + 512].T)
    kT2 = np.ascontiguousarray(k[:, 1024 * s:1024 * s + 1024].T)
    kB1 = np.ascontiguousarray(k[:, 256 * s:256 * s + 256].T)
    arrs = {
        "qA1": f16(qT1),
        "qB1s": f16(_sigma(qT1, 32, 64)),
        "qA2": f16(qT2), "qB2s": f16(_sigma(qT2, 64, 128)),
        "kA1": f16(kT1),
        "kA2": f16(kT2),
        "kB1": f16(kB1), "kB1s": f16(_sigma(kB1, 32, 64)),
        "vA1": f16(v[:, 512 * s:512 * s + 512] / 16.0),
        "vA2": f16(v[:, 1024 * s:1024 * s + 1024] / 16.0),
        "vB1": f16(v[:, 256 * s:256 * s + 256] / 16.0),
    }
    fA = math.sqrt(1.0 / 16.0)
    fB = math.sqrt(1.0 / math.sqrt(128.0))
    ca1, sa1 = _rope_tab(pos, 128, fA * float(weights[0]))
    ca2, sa2 = _rope_tab(pos, 256, fA * float(weights[1]))
    cb1h, sb1h = _rope_tab(pos, 64, fB * float(weights[2]))
    cb2, sb2 = _rope_tab(pos, 128, fB * float(weights[3]))
    arrs.update({
        "ca1": f16(ca1), "sa1": f16(sa1),
        "ca2": f16(ca2), "sa2": f16(sa2),
        "cb1": f16(np.vstack([cb1h, cb1h])), "sb1": f16(np.vstack([sb1h, sb1h])),
        "cb2": f16(cb2), "sb2": f16(sb2),
        # pt is scale-invariant under softmax: fold w1 (w3) into the den
        # "ones" columns and keep only the w0/w1 (w2/w3) ratio on the v1 add
        "wr0": f16(np.full((P, 512), float(weights[0] / weights[1]))),
        "wr2": f16(np.full((P, 256), float(weights[2] / weights[3]))),
        "onesA": f16(np.full((P, 128), float(1.0 / (16.0 * weights[1])))),
        "onesB": f16(np.full((P, 128), float(1.0 / (16.0 * weights[3])))),
    })
    return arrs


_PROGRAM_CACHE = {}
TRACE = False
LAST_RESULT = None
NEG = -1e9


def kernel(q_m, k_m, v_m, weights, attention_mask, position_ids):
    global LAST_RESULT
    from concourse.bass_utils import run_bass_kernel_spmd

    cfg = FULL
    q_m = np.asarray(q_m, np.float32)
    k_m = np.asarray(k_m, np.float32)
    v_m = np.asarray(v_m, np.float32)
    weights = np.asarray(weights, np.float32)
    attention_mask = np.asarray(attention_mask, np.float32)
    position_ids = np.asarray(position_ids)
    B, T, H = q_m.shape

    # the device program hardcodes the causal structure; verify it holds
    causal = np.where(np.tril(np.ones((T, T), bool)), 0.0, NEG).astype(np.float32)
    for b in range(B):
        assert np.array_equal(attention_mask[b, 0], causal), "non-causal mask"

    if "nc" not in _PROGRAM_CACHE:
        _PROGRAM_CACHE["nc"] = build_program(cfg)
    nc = _PROGRAM_CACHE["nc"]

    in_maps = []
    for b in range(B):
        for s in range(2):
            in_maps.append(make_core_inputs(
                q_m[b], k_m[b], v_m[b], position_ids[b], weights, s, cfg))
    res = run_bass_kernel_spmd(nc, in_maps, list(range(8)), trace=TRACE)
    LAST_RESULT = res
    out = np.zeros((B, T, H), np.float32)
    for b in range(B):
        for s in range(2):
            out[b, :, 1024 * s:1024 * s + 1024] = \
                res.results[2 * b + s]["outT"].astype(np.float32).T
    return out


# revision 74
# speedup vs baseline: 1.8361x; 1.0082x over previous
"""Trainium2 Bass kernel for nn_MixedAttnHeadEmbed (mixed-head-config attention).

Math (per batch b):
  Two attention configs share q_m/k_m/v_m [B,T,2048]:
    A: h=8  heads, d_max=256, mixing e in {1024,2048} -> d in {128,256}, weights w0,w1
    B: h=16 heads, d_max=128, mixing e in {1024,2048} -> d in {64,128},  weights w2,w3
  Each config: per-head q/k slices are RoPE'd, weight-summed (padded to d_max),
  GQA (8 kv heads), causal softmax attention; outputs of both configs sum.

Sharding: 8 cores = 4 batches x 2 shards. Shard s owns A-heads [4s,4s+4) and
B-heads [8s,8s+8) -> both write output columns [1024s, 1024s+1024) which are
summed on device; per-core output is the transposed block outT [1024, T] fp16.

Device design (driven by the CoreSim cost model):
  * All on-device data is fp16 (PSUM accumulation stays fp32): DVE gets the
    2x fast mode for 2-byte dtypes, the PE runs 1 col/cycle at any moving
    width (f32r pays 4x under 256 cols), and DMA bytes halve. fp16's 5e-4
    epsilon keeps the end-to-end error ~1e-3, far under the 2e-2 gate.
  * RoPE rotations are eliminated on device: the host uploads sigma-permuted
    row copies of each q/k slice (rows swapped within each rotation group),
    so rope+mix is a chain of partition-aligned tensor_tensor ops against
    sign-folded sin/cos tables (weights and 1/sqrt(d) folded in on host).
  * Scores are computed transposed (sT[k,q], k on partitions) so softmax'd
    weights feed the y^T matmul with no transposes; softmax is max-free
    (scores provably < 2) with the denominator from an all-ones matmul.
  * Causal diag-block masking zeroes pt after the exp via a Pool
    affine_select instead of adding a mask into PSUM on DVE.
  * The PE stream is software-pipelined: scores for k-chunk c+1 are issued
    before y/den for chunk c, so the PE does not sit behind each exp.
"""

import math
from contextlib import ExitStack
from dataclasses import dataclass

import numpy as np

import concourse.bass as bass
import concourse.mybir as mybir
import concourse.tile as tile
from concourse import bacc

F32 = mybir.dt.float32
PTA_BUFS, PTB_BUFS, LAG_A, LAG_B = 10, 7, 4, 5
MID_C = 2
F16 = mybir.dt.float16
P = 128


@dataclass(frozen=True)
class KCfg:
    T: int = 1024       # sequence length
    NA: int = 4         # config-A heads per core (d_max=256)
    NB: int = 8         # config-B heads per core (d_max=128)
    REG: int = 512      # psum region width

    @property
    def TK(self):
        return self.T // P

    @property
    def NREG(self):
        return self.T // self.REG


FULL = KCfg()


def _in_specs(cfg: KCfg):
    T = cfg.T
    return {
        # q/k transposed slices (rows = head dims, fp16). *s = sigma-permuted
        # rows (rotation pairing partner), so rope is all aligned TT ops.
        "qA1": (512, T), "qB1s": (512, T),
        "qA2": (1024, T), "qB2s": (1024, T),
        "kA1": (512, T),
        "kA2": (1024, T),
        "kB1": (256, T), "kB1s": (256, T),
        # v slices, natural [T, d] layout
        "vA1": (T, 512), "vA2": (T, 1024), "vB1": (T, 256),
        # rope tables [d, T], weights+scale folded, sin sign-folded
        "ca1": (128, T), "sa1": (128, T),
        "ca2": (256, T), "sa2": (256, T),
        "cb1": (128, T), "sb1": (128, T),
        "cb2": (128, T), "sb2": (128, T),
        # v mixing: ratio rows (w0/w1, w2/w3) and 1/w den-ones columns
        "wr0": (P, 512), "wr2": (P, 256),
        "onesA": (P, 128), "onesB": (P, 128),
    }


class _Pick:
    """Static DVE-vs-Pool load balancer with cost-model-accurate weights.

    DVE: n*0.5208ns fp16 TT (2x mode), n*0.26 fp16 copy (4x), n*1.0417
    for psum/fp32 ops (+60/+125ns access). Pool: n*0.8333 flat. ACT is
    reserved for the exps (it is the 2nd-busiest engine)."""

    def __init__(self, nc):
        self.nc = nc
        self.load = {"dve": 0.0, "pool": 0.0}

    def _eng(self, cd, cp):
        if self.load["dve"] + cd <= self.load["pool"] + cp:
            self.load["dve"] += cd
            return self.nc.vector
        self.load["pool"] += cp
        return self.nc.gpsimd

    def tt16(self, out, in0, in1, op, n):
        e = self._eng(n * 0.5208 + 60, n * 0.8333 + 25)
        e.tensor_tensor(out, in0, in1, op)

    def recip(self, out, in_, n):
        """PSUM->SBUF reciprocal: DVE only (Pool has no PSUM port)."""
        self.load["dve"] += n * 1.0417 + 125
        self.nc.vector.reciprocal(out, in_)

    def dve_psmul(self, out, in0, in1, n):
        """TT mult with one PSUM f32 operand: DVE only."""
        self.load["dve"] += n * 1.0417 + 125
        self.nc.vector.tensor_tensor(out, in0, in1, mybir.AluOpType.mult)

    def act_exp(self, out, in_, n):
        self.load["act"] = self.load.get("act", 0.0) + n * 0.8333 + 185
        self.nc.scalar.activation(out, in_, mybir.ActivationFunctionType.Exp)

    def copy_ps(self, dst, src, n, on_act):
        """PSUM->SBUF copy. Phase-aware placement: ACT idles during config A
        (PE-bound) but saturates during config B, where DVE/Pool idle --
        so A-head copies go to ACT ('copy' shares the exp table, no reload)
        and B-head copies go to DVE."""
        if on_act:
            self.load["act"] = self.load.get("act", 0.0) + n * 0.8333 + 185
            self.nc.scalar.copy(dst, src)
        else:
            self.load["dve"] += n * 1.0417 + 125
            self.nc.vector.tensor_copy(dst, src)

    def copy16(self, dst, src, n):
        e = self._eng(n * 0.26 + 60, n * 0.8333 + 25)
        e.tensor_copy(dst, src)

    def pool_fix(self, n):
        self.load["pool"] += n * 0.8333 + 25
        return self.nc.gpsimd


def build_program(cfg: KCfg = FULL):
    nc = bacc.Bacc("TRN2", target_bir_lowering=False)
    T, TK, REG, NREG = cfg.T, cfg.TK, cfg.REG, cfg.NREG
    RPB = REG // P

    D = {}
    for name, shape in _in_specs(cfg).items():
        D[name] = nc.declare_dram_parameter(name, list(shape), F16, isOutput=False)
    outT = nc.declare_dram_parameter("outT", [1024, T], F16, isOutput=True)

    mult, add = mybir.AluOpType.mult, mybir.AluOpType.add

    def subchunks(c):
        out = []
        for r in range(NREG):
            q0 = max(REG * r, P * c)
            q1 = REG * (r + 1)
            if q1 > q0:
                out.append((r, q0, q1 - q0))
        return out

    last_c = [min(TK, RPB * (r + 1)) - 1 for r in range(NREG)]

    with ExitStack() as ctx:
        tc = ctx.enter_context(tile.TileContext(nc))
        const = ctx.enter_context(tc.tile_pool(name="const", bufs=1))
        rawq = ctx.enter_context(tc.tile_pool(name="rawq", bufs=1))
        vmixp = ctx.enter_context(tc.tile_pool(name="vmix", bufs=1))
        vraw = ctx.enter_context(tc.tile_pool(name="vraw", bufs=1))
        accp = ctx.enter_context(tc.tile_pool(name="acc", bufs=1))
        x2p = ctx.enter_context(tc.tile_pool(name="x2p", bufs=2))
        mixp = ctx.enter_context(tc.tile_pool(name="mix", bufs=2))
        scr = ctx.enter_context(tc.tile_pool(name="scr", bufs=2))
        normp = ctx.enter_context(tc.tile_pool(name="norm", bufs=2))
        pta = ctx.enter_context(tc.tile_pool(name="pta", bufs=PTA_BUFS))
        ptb = ctx.enter_context(tc.tile_pool(name="ptb", bufs=PTB_BUFS))

        pick = _Pick(nc)

        # DMAs are emitted in first-use order (the DMA device is serial in
        # the model): A tables -> head-0 slices -> v chunks -> later heads ->
        # B-only tables/slices. Raw q/k persistent tiles are filled by
        # per-head slice DMAs so head 0's data lands first.
        onesA = const.tile([P, P], F16)
        onesB = const.tile([P, P], F16)
        nc.sync.dma_start(out=onesA, in_=D["onesA"][:, :])
        nc.sync.dma_start(out=onesB, in_=D["onesB"][:, :])
        tabs = {}

        def load_tab(nm):
            rows = _in_specs(cfg)[nm][0]
            tl = const.tile([P, rows // P, T], F16, name=nm, tag=nm)
            tabs[nm] = tl
            nc.sync.dma_start(out=tl, in_=D[nm].rearrange("(c p) t -> p c t", p=P))

        wrs = {}

        def load_wrs():
            for nm in ("wr0", "wr2"):
                cols = _in_specs(cfg)[nm][1]
                tl = const.tile([P, cols], F16, name=nm, tag=nm)
                wrs[nm] = tl
                nc.sync.dma_start(out=tl, in_=D[nm][:, :])

        raw = {}
        for nm in ("qA1", "qB1s", "kA1", "kB1", "kB1s"):
            rows = _in_specs(cfg)[nm][0]
            raw[nm] = rawq.tile([P, rows // P, T], F16, name=nm, tag=nm)
        for nm in ("qA1s", "kA1s"):  # device-built sigma64 copies
            raw[nm] = rawq.tile([P, 4, T], F16, name=nm, tag=nm)

        def emit_sigma64(nm, h):
            s_, d_ = raw[nm[:3]], raw[nm]
            pick.copy16(d_[0:64, h, :], s_[64:128, h, :], 1024)
            pick.copy16(d_[64:128, h, :], s_[0:64, h, :], 1024)

        def load_raw_slice(nm, i):
            nc.sync.dma_start(out=raw[nm][:, i, :],
                              in_=D[nm][P * i:P * (i + 1), :])

        def load_x2(dram, pair, tag):
            tl = x2p.tile([P, 2, T], F16, tag=tag, name=tag)
            for i in range(2):  # split halves: part 0 usable earlier
                nc.sync.dma_start(
                    out=tl[:, i, :],
                    in_=dram[P * (2 * pair + i):P * (2 * pair + i + 1), :])
            return tl

        # head-0 loads, interleaved with the tables each mix op needs so the
        # first products can start ~4us in; wrs (v-mix weights) come last
        q2_0 = x2p.tile([P, 2, T], F16, tag="x2a", name="x2a")
        k2_0 = x2p.tile([P, 2, T], F16, tag="x2b", name="x2b")
        nc.sync.dma_start(out=q2_0[:, 0, :], in_=D["qA2"][0:P, :])
        load_tab("ca2")
        nc.sync.dma_start(out=q2_0[:, 1, :], in_=D["qA2"][P:2 * P, :])
        load_tab("sa2")
        nc.sync.dma_start(out=k2_0[:, 0, :], in_=D["kA2"][0:P, :])
        nc.sync.dma_start(out=k2_0[:, 1, :], in_=D["kA2"][P:2 * P, :])
        load_raw_slice("kA1", 0)
        load_raw_slice("qA1", 0)
        load_tab("ca1")
        load_tab("sa1")
        load_wrs()

        # v tiles; vA2 is DMA'd straight into vmixA and scaled in place.
        # Chunk DMAs are interleaved with head-1's loads further down so the
        # serial DMA queue tracks consumption order.
        vmixA = vmixp.tile([P, TK, 4, 256], F16)
        vmixB = vmixp.tile([P, TK, 4, 128], F16)
        vB1r = vraw.tile([P, TK, 256], F16)

        def load_v_chunk(c):
            nc.sync.dma_start(out=vmixA[:, c],
                              in_=D["vA2"][P * c:P * (c + 1), :])
            nc.sync.dma_start(out=vmixB[:, c],
                              in_=D["vA1"][P * c:P * (c + 1), :])

        # pt is softmax-scale-invariant, so w1/w3 live in the den "ones"
        # columns: vmixA = vA2 + (w0/w1)*vA1 pad; vmixB = vA1 + (w2/w3)*vB1.
        # vmixA's ratio-mult reads vmixB (raw vA1) BEFORE emit_vmixB's add
        # modifies its first 64 cols per kv -- emission order guarantees it.

        def emit_vmixA():
            for c in range(TK):
                vt1 = scr.tile([P, 4, 128], F16, tag="mt2")
                pick.tt16(vt1, vmixB[:, c, :], wrs["wr0"], mult, 512)
                pick.tt16(vmixA[:, c, :, 0:128], vmixA[:, c, :, 0:128], vt1,
                          add, 512)

        def emit_vmixB():
            for c in range(TK):
                vt2 = scr.tile([P, 4, 64], F16, tag="mu2")
                pick.tt16(vt2, vB1r[:, c, :], wrs["wr2"], mult, 256)
                pick.tt16(vmixB[:, c, :, 0:64], vmixB[:, c, :, 0:64], vt2,
                          add, 256)

        outacc = accp.tile([P, 8, T], F16)

        def mix_A_dc0(dst, x1, x1s, x2):
            """dst[:,0,:] = (x2_0*c2_0 + x2_1*s2_0) + (x1*c1 + x1s*s1)."""
            t = scr.tile([P, T], F16, tag="mt")
            u = scr.tile([P, T], F16, tag="mu")
            t2 = scr.tile([P, T], F16, tag="mt2")
            u2 = scr.tile([P, T], F16, tag="mu2")
            pick.tt16(t, x2[:, 0, :], tabs["ca2"][:, 0, :], mult, 1024)
            pick.tt16(u, x2[:, 1, :], tabs["sa2"][:, 0, :], mult, 1024)
            pick.tt16(t2, x1, tabs["ca1"][:, 0, :], mult, 1024)
            pick.tt16(u2, x1s, tabs["sa1"][:, 0, :], mult, 1024)
            pick.tt16(t, t, u, add, 1024)
            pick.tt16(t2, t2, u2, add, 1024)
            pick.tt16(dst[:, 0, :], t, t2, add, 1024)

        def mix_A_dc1(dst, x2):
            """dst[:,1,:] = x2_1*c2_1 + x2_0*s2_1."""
            t = scr.tile([P, T], F16, tag="mt")
            u = scr.tile([P, T], F16, tag="mu")
            pick.tt16(t, x2[:, 1, :], tabs["ca2"][:, 1, :], mult, 1024)
            pick.tt16(u, x2[:, 0, :], tabs["sa2"][:, 1, :], mult, 1024)
            pick.tt16(dst[:, 1, :], t, u, add, 1024)

        def mix_B_pair(dst, x2, x2s, x1, x1s):
            """dst [P,2,T]: B-heads (2j,2j+1) rope mix.
            x2/x2s [P,2,T] d128 nat/sigma64; x1/x1s [P,T] packed d64 pair."""
            for hh in range(2):
                t2 = scr.tile([P, T], F16, tag="mt")
                u2 = scr.tile([P, T], F16, tag="mu")
                pick.tt16(t2, x2[:, hh, :], tabs["cb2"][:, 0, :], mult, 1024)
                pick.tt16(u2, x2s[:, hh, :], tabs["sb2"][:, 0, :], mult, 1024)
                pick.tt16(dst[:, hh, :], t2, u2, add, 1024)
            # packed d64 part for both heads
            t = scr.tile([P, T], F16, tag="mt2")
            u = scr.tile([P, T], F16, tag="mu2")
            pick.tt16(t, x1, tabs["cb1"][:, 0, :], mult, 1024)
            pick.tt16(u, x1s, tabs["sb1"][:, 0, :], mult, 1024)
            pick.tt16(t, t, u, add, 1024)
            # head 2j: rows 0:64 aligned; head 2j+1: cross-base copy first
            pick.tt16(dst[0:64, 0, :], dst[0:64, 0, :], t[0:64, :], add, 1024)
            pick.copy16(u[0:64, :], t[64:128, :], 1024)
            pick.tt16(dst[0:64, 1, :], dst[0:64, 1, :], u[0:64, :], add, 1024)

        def attn_head(qmix_dcs, kmix_dcs, vmix_ap, blks, spool, ypool, dpool,
                      wide_sT, is_b, lag=1, mid_emit=None):
            ones = onesB if is_b else onesA
            """qmix_dcs/kmix_dcs: list of [P,T] APs (one per 128-d chunk).
            vmix_ap(c, dc) -> stationary [P,128]. blks: output block ids.
            Emits PE work software-pipelined: scores(c) ... y/den(c-lag)."""
            ndc = len(qmix_dcs)
            den = dpool.tile([P, T], F32, tag="den")
            yts = [ypool.tile([P, T], F32, tag=f"yt{i}", name=f"yt{i}")
                   for i in range(ndc)]
            pending = []  # deferred (c, parts) lists, flushed `lag` behind

            def emit_norm(r):
                # region r's den/y accumulation is complete: normalize and
                # (for B) add+store now, overlapping the remaining chunks.
                # rec stays in fp16's normal range: the host pre-scales v and
                # the ones columns by 1/16 (softmax is pt-scale-invariant).
                sl = slice(REG * r, REG * (r + 1))
                rec = normp.tile([P, REG], F16, tag="rec")
                with nc.allow_low_precision(
                        reason="softmax rec in fp16; host pre-scales 1/16 "
                               "so rec stays in fp16 normal range"):
                    pick.recip(rec, den[:, sl], REG)
                for dc in range(ndc):
                    blk = blks[dc]
                    y_sb = normp.tile([P, REG], F16, tag="ysb")
                    pick.copy_ps(y_sb, yts[dc][:, sl], REG, on_act=not is_b)
                    if not is_b:
                        pick.tt16(outacc[:, blk, sl], y_sb, rec, mult, 512)
                    else:
                        tmp = normp.tile([P, REG], F16, tag="btmp")
                        pick.tt16(tmp, y_sb, rec, mult, 512)
                        pick.tt16(outacc[:, blk, sl], outacc[:, blk, sl],
                                  tmp, add, 512)
                        nc.sync.dma_start(
                            out=outT[P * blk:P * (blk + 1), sl],
                            in_=outacc[:, blk, sl])

            def flush(pend):
                c, parts = pend
                for (q0, n, pt, off) in parts:
                    r = q0 // REG
                    # the diagonal 128 cols wait on the affine mask; split
                    # them off so the bulk matmuls only depend on the exp.
                    # (not at c==0: two start=True groups in one PSUM zero
                    # region are illegal)
                    segs = [(q0, n, off)]
                    if q0 == P * c and n > P and c > 0:
                        segs = [(q0 + P, n - P, off + P), (q0, P, off)]
                    for (s0, sn, so) in segs:
                        for dc in range(ndc):
                            nc.tensor.matmul(
                                yts[dc][:, s0:s0 + sn], vmix_ap(c, dc),
                                pt[:, so:so + sn],
                                start=(c == 0), stop=(c == last_c[r]))
                        nc.tensor.matmul(den[:, s0:s0 + sn], ones,
                                         pt[:, so:so + sn],
                                         start=(c == 0), stop=(c == last_c[r]))
                for r in range(NREG):
                    if last_c[r] == c:
                        emit_norm(r)

            for c in range(TK):
                if c == MID_C and mid_emit is not None:
                    mid_emit()
                parts = []
                if wide_sT:
                    # one [P,T] sT + one exp for the whole chunk
                    sT = spool.tile([P, T], F32, tag="sTw")
                    pt = ptb.tile([P, T], F16, tag="ptw")
                    for (r, q0, n) in subchunks(c):
                        for dc in range(ndc):
                            nc.tensor.matmul(
                                sT[:, q0:q0 + n],
                                kmix_dcs[dc][:, P * c:P * (c + 1)],
                                qmix_dcs[dc][:, q0:q0 + n],
                                start=(dc == 0), stop=(dc == ndc - 1))
                    pick.act_exp(pt[:, P * c:], sT[:, P * c:], T - P * c)
                    parts = [(q0, n, pt, q0) for (r, q0, n) in subchunks(c)]
                    diag = (pt, P * c)
                else:
                    # [P,REG] sT + exp per subchunk
                    diag = None
                    for (r, q0, n) in subchunks(c):
                        sT = spool.tile([P, REG], F32, tag="sTn")
                        pt = pta.tile([P, REG], F16, tag="ptn")
                        for dc in range(ndc):
                            nc.tensor.matmul(
                                sT[:, 0:n],
                                kmix_dcs[dc][:, P * c:P * (c + 1)],
                                qmix_dcs[dc][:, q0:q0 + n],
                                start=(dc == 0), stop=(dc == ndc - 1))
                        pick.act_exp(pt[:, 0:n], sT[:, 0:n], n)
                        if q0 == P * c:
                            diag = (pt, 0)
                        parts.append((q0, n, pt, 0))
                # zero the masked (k>q) half of the causal diagonal block
                dpt, doff = diag
                pick.pool_fix(128).affine_select(
                    out=dpt[:, doff:doff + P], in_=dpt[:, doff:doff + P],
                    compare_op=mybir.AluOpType.is_ge, fill=0.0,
                    base=0, pattern=[[1, P]], channel_multiplier=-1)
                pending.append((c, parts))
                if len(pending) > lag:
                    flush(pending.pop(0))
            for pend in pending:
                flush(pend)

        def mix_A_head(h, q2=None, k2=None):
            # DMAs emitted here so the serial DMA queue runs in use order;
            # dc0 chains for q AND k first so the first score matmul (which
            # only needs dc0 of both) is unblocked as early as possible.
            if q2 is None:
                q2 = load_x2(D["qA2"], h, "x2a")
                load_raw_slice("qA1", h)
                k2 = load_x2(D["kA2"], h, "x2b")
                load_raw_slice("kA1", h)
            emit_sigma64("qA1s", h)
            emit_sigma64("kA1s", h)
            qmix = mixp.tile([P, 2, T], F16, tag="qmix")
            kmix = mixp.tile([P, 2, T], F16, tag="kmix")
            mix_A_dc0(qmix, raw["qA1"][:, h, :], raw["qA1s"][:, h, :], q2)
            mix_A_dc0(kmix, raw["kA1"][:, h, :], raw["kA1s"][:, h, :], k2)
            mix_A_dc1(qmix, q2)
            mix_A_dc1(kmix, k2)
            return qmix, kmix

        def mix_B_group(j):
            """kv j's kmix + the q pair for heads (2j, 2j+1)."""
            if j == 0:
                for nm in ("cb2", "sb2", "cb1", "sb1"):
                    load_tab(nm)
            if j % 2 == 0:
                load_raw_slice("kB1", j // 2)
                load_raw_slice("kB1s", j // 2)
            load_raw_slice("qB1s", j)
            kmix = mixp.tile([P, T], F16, tag="kmixB")
            t = scr.tile([P, T], F16, tag="mt2")
            u = scr.tile([P, T], F16, tag="mu2")
            pick.tt16(t, raw["kA1"][:, j, :], tabs["cb2"][:, 0, :], mult, 1024)
            pick.tt16(u, raw["kA1s"][:, j, :], tabs["sb2"][:, 0, :], mult, 1024)
            pick.tt16(kmix, t, u, add, 1024)
            # d64 part: computed once per kv pair (kB1 tile j//2), reused by
            # the odd kv via a cross-base copy
            jp, g = j // 2, j % 2
            if g == 0:
                t64 = scr.tile([P, T], F16, tag="t64")
                pick.tt16(t64, raw["kB1"][:, jp, :], tabs["cb1"][:, 0, :],
                          mult, 1024)
                pick.tt16(u, raw["kB1s"][:, jp, :], tabs["sb1"][:, 0, :],
                          mult, 1024)
                pick.tt16(t64, t64, u, add, 1024)
                st["t64"] = t64
                pick.tt16(kmix[0:64, :], kmix[0:64, :], t64[0:64, :], add, 1024)
            else:
                pick.copy16(u[0:64, :], st["t64"][64:128, :], 1024)
                pick.tt16(kmix[0:64, :], kmix[0:64, :], u[0:64, :], add, 1024)

            q2 = load_x2(D["qA2"], j, "x2a")
            q2s = load_x2(D["qB2s"], j, "x2b")
            qmixp = mixp.tile([P, 2, T], F16, tag="qmix")
            mix_B_pair(qmixp, q2, q2s,
                       raw["qA1"][:, j, :], raw["qB1s"][:, j, :])
            return qmixp, kmix

        # ============ config A (B group 0's mix overlaps A3's attn) ========
        st = {"amix": mix_A_head(0, q2_0, k2_0), "bmix": None}
        # interleave v-chunk DMAs with head 1's loads in the serial DMA queue
        load_v_chunk(0)
        load_v_chunk(1)
        q2_1 = load_x2(D["qA2"], 1, "x2a")
        load_raw_slice("qA1", 1)
        load_v_chunk(2)
        load_v_chunk(3)
        k2_1 = load_x2(D["kA2"], 1, "x2b")
        load_raw_slice("kA1", 1)
        for c in range(4, TK):
            load_v_chunk(c)
        emit_vmixA()

        with tc.tile_pool(name="spsA", bufs=2, space="PSUM") as spsA, \
             tc.tile_pool(name="ypsA", bufs=1, space="PSUM") as ypsA, \
             tc.tile_pool(name="dpsA", bufs=1, space="PSUM") as dpsA:
            for h in range(cfg.NA):
                qmix, kmix = st["amix"]

                def filler(h=h):
                    # vB1/vmixB wait until h==2 so heads 2-3's DMA bundles
                    # are not queued behind them (vmixB is first used by B0)
                    if h == 0:
                        st["amix"] = mix_A_head(1, q2_1, k2_1)
                    elif h == 1:
                        st["amix"] = mix_A_head(2)
                    elif h == 2:
                        st["amix"] = mix_A_head(3)
                        for c in range(TK):
                            nc.sync.dma_start(out=vB1r[:, c, :],
                                              in_=D["vB1"][P * c:P * (c + 1), :])
                        emit_vmixB()
                    else:
                        st["bmix"] = mix_B_group(0)

                def vap(c, dc, h=h):
                    return vmixA[:, c, h, 128 * dc:128 * (dc + 1)]

                attn_head([qmix[:, 0, :], qmix[:, 1, :]],
                          [kmix[:, 0, :], kmix[:, 1, :]],
                          vap, (2 * h, 2 * h + 1), spsA, ypsA, dpsA,
                          wide_sT=False, is_b=False, lag=LAG_A, mid_emit=filler)

        # ================= config B =================
        with tc.tile_pool(name="spsB", bufs=2, space="PSUM") as spsB, \
             tc.tile_pool(name="ypsB", bufs=1, space="PSUM") as ypsB, \
             tc.tile_pool(name="dpsB", bufs=1, space="PSUM") as dpsB:
            for j in range(4):  # kv head j serves B-heads (2j, 2j+1)
                qmixp, kmix = st["bmix"]

                def vapB(c, dc, j=j):
                    return vmixB[:, c, j, :]

                for hh in range(2):
                    b = 2 * j + hh
                    filler = None
                    if hh == 1 and j + 1 < 4:
                        def filler(j=j):
                            st["bmix"] = mix_B_group(j + 1)
                    attn_head([qmixp[:, hh, :]], [kmix], vapB, (b,),
                              spsB, ypsB, dpsB, wide_sT=True, is_b=True,
                              lag=LAG_B, mid_emit=filler)

    nc.compile()
    return nc


# ---------------------------------------------------------------------------
# Host side
# ---------------------------------------------------------------------------

def _rope_tab(pos, d, f):
    """Rope tables [d, T]: (f*cos, signed f*sin); sign folded so that
    out[j] = x[j]*c[j] + x[sigma(j)]*s[j] with sigma the half-swap."""
    inv = 1.0 / (10000.0 ** (np.arange(0, d, 2, dtype=np.float32) / d))
    ang = inv[:, None] * pos[None, :].astype(np.float32)      # [d/2, T]
    ang = np.concatenate([ang, ang], 0)                        # [d, T]
    c = (f * np.cos(ang)).astype(np.float32)
    s = (f * np.sin(ang)).astype(np.float32)
    s[: d // 2] *= -1.0
    return c, s


def _sigma(x, half, group):
    """Swap half-blocks of `half` rows within each `group`-row group."""
    r = x.reshape(-1, 2, half, x.shape[-1])
    assert group == 2 * half
    return np.ascontiguousarray(r[:, ::-1].reshape(x.shape))


def make_core_inputs(q, k, v, pos, weights, s, cfg: KCfg = FULL):
    """q,k,v: [T, 2048] fp32 for one batch; returns per-core input dict."""
    f16 = lambda a: np.ascontiguousarray(a, dtype=np.float16)
    qT1 = np.ascontiguousarray(q[:, 512 * s:512 * s + 512].T)
    qT2 = np.ascontiguousarray(q[:, 1024 * s:1024 * s + 1024].T)
    kT1 = np.ascontiguousarray(k[:, 512 * s:512 * s + 512].T)
    kT2 = np.ascontiguousarray(k[:, 1024 * s:1024 * s + 1024].T)
    kB1 = np.ascontiguousarray(k[:, 256 * s:256 * s + 256].T)
    arrs = {
        "qA1": f16(qT1),
        "qB1s": f16(_sigma(qT1, 32, 64)),
        "qA2": f16(qT2), "qB2s": f16(_sigma(qT2, 64, 128)),
        "kA1": f16(kT1),
        "kA2": f16(kT2),
        "kB1": f16(kB1), "kB1s": f16(_sigma(kB1, 32, 64)),
        "vA1": f16(v[:, 512 * s:512 * s + 512] / 16.0),
        "vA2": f16(v[:, 1024 * s:1024 * s + 1024] / 16.0),
        "vB1": f16(v[:, 256 * s:256 * s + 256] / 16.0),
    }
    fA = math.sqrt(1.0 / 16.0)
    fB = math.sqrt(1.0 / math.sqrt(128.0))
    ca1, sa1 = _rope_tab(pos, 128, fA * float(weights[0]))
    ca2, sa2 = _rope_tab(pos, 256, fA * float(weights[1]))
    cb1h, sb1h = _rope_tab(pos, 64, fB * float(weights[2]))
    cb2, sb2 = _rope_tab(pos, 128, fB * float(weights[3]))
    arrs.update({
        "ca1": f16(ca1), "sa1": f16(sa1),
        "ca2": f16(ca2), "sa2": f16(sa2),
        "cb1": f16(np.vstack([cb1h, cb1h])), "sb1": f16(np.vstack([sb1h, sb1h])),
        "cb2": f16(cb2), "sb2": f16(sb2),
        # pt is scale-invariant under softmax: fold w1 (w3) into the den
        # "ones" columns and keep only the w0/w1 (w2/w3) ratio on the v1 add
        "wr0": f16(np.full((P, 512), float(weights[0] / weights[1]))),
        "wr2": f16(np.full((P, 256), float(weights[2] / weights[3]))),
        "onesA": f16(np.full((P, 128), float(1.0 / (16.0 * weights[1])))),
        "onesB": f16(np.full((P, 128), float(1.0 / (16.0 * weights[3])))),
    })
    return arrs


_PROGRAM_CACHE = {}
TRACE = False
LAST_RESULT = None
NEG = -1e9


def kernel(q_m, k_m, v_m, weights, attention_mask, position_ids):
    global LAST_RESULT
    from concourse.bass_utils import run_bass_kernel_spmd

    cfg = FULL
    q_m = np.asarray(q_m, np.float32)
    k_m = np.asarray(k_m, np.float32)
    v_m = np.asarray(v_m, np.float32)
    weights = np.asarray(weights, np.float32)
    attention_mask = np.asarray(attention_mask, np.float32)
    position_ids = np.asarray(position_ids)
    B, T, H = q_m.shape

    # the device program hardcodes the causal structure; verify it holds
    causal = np.where(np.tril(np.ones((T, T), bool)), 0.0, NEG).astype(np.float32)
    for b in range(B):
        assert np.array_equal(attention_mask[b, 0], causal), "non-causal mask"

    if "nc" not in _PROGRAM_CACHE:
        _PROGRAM_CACHE["nc"] = build_program(cfg)
    nc = _PROGRAM_CACHE["nc"]

    in_maps = []
    for b in range(B):
        for s in range(2):
            in_maps.append(make_core_inputs(
                q_m[b], k_m[b], v_m[b], position_ids[b], weights, s, cfg))
    res = run_bass_kernel_spmd(nc, in_maps, list(range(8)), trace=TRACE)
    LAST_RESULT = res
    out = np.zeros((B, T, H), np.float32)
    for b in range(B):
        for s in range(2):
            out[b, :, 1024 * s:1024 * s + 1024] = \
                res.results[2 * b + s]["outT"].astype(np.float32).T
    return out


# revision 76
# speedup vs baseline: 1.8447x; 1.0047x over previous
"""Trainium2 Bass kernel for nn_MixedAttnHeadEmbed (mixed-head-config attention).

Math (per batch b):
  Two attention configs share q_m/k_m/v_m [B,T,2048]:
    A: h=8  heads, d_max=256, mixing e in {1024,2048} -> d in {128,256}, weights w0,w1
    B: h=16 heads, d_max=128, mixing e in {1024,2048} -> d in {64,128},  weights w2,w3
  Each config: per-head q/k slices are RoPE'd, weight-summed (padded to d_max),
  GQA (8 kv heads), causal softmax attention; outputs of both configs sum.

Sharding: 8 cores = 4 batches x 2 shards. Shard s owns A-heads [4s,4s+4) and
B-heads [8s,8s+8) -> both write output columns [1024s, 1024s+1024) which are
summed on device; per-core output is the transposed block outT [1024, T] fp16.

Device design (driven by the CoreSim cost model):
  * All on-device data is fp16 (PSUM accumulation stays fp32): DVE gets the
    2x fast mode for 2-byte dtypes, the PE runs 1 col/cycle at any moving
    width (f32r pays 4x under 256 cols), and DMA bytes halve. fp16's 5e-4
    epsilon keeps the end-to-end error ~1e-3, far under the 2e-2 gate.
  * RoPE rotations are eliminated on device: the host uploads sigma-permuted
    row copies of each q/k slice (rows swapped within each rotation group),
    so rope+mix is a chain of partition-aligned tensor_tensor ops against
    sign-folded sin/cos tables (weights and 1/sqrt(d) folded in on host).
  * Scores are computed transposed (sT[k,q], k on partitions) so softmax'd
    weights feed the y^T matmul with no transposes; softmax is max-free
    (scores provably < 2) with the denominator from an all-ones matmul.
  * Causal diag-block masking zeroes pt after the exp via a Pool
    affine_select instead of adding a mask into PSUM on DVE.
  * The PE stream is software-pipelined: scores for k-chunk c+1 are issued
    before y/den for chunk c, so the PE does not sit behind each exp.
"""

import math
from contextlib import ExitStack
from dataclasses import dataclass

import numpy as np

import concourse.bass as bass
import concourse.mybir as mybir
import concourse.tile as tile
from concourse import bacc

F32 = mybir.dt.float32
PTA_BUFS, PTB_BUFS, LAG_A, LAG_B = 10, 7, 4, 5
MID_C = 2
MID_CB = 2
SCR_BUFS, NORM_BUFS, X2_BUFS, MIX_BUFS = 2, 3, 2, 2
F16 = mybir.dt.float16
P = 128


@dataclass(frozen=True)
class KCfg:
    T: int = 1024       # sequence length
    NA: int = 4         # config-A heads per core (d_max=256)
    NB: int = 8         # config-B heads per core (d_max=128)
    REG: int = 512      # psum region width

    @property
    def TK(self):
        return self.T // P

    @property
    def NREG(self):
        return self.T // self.REG


FULL = KCfg()


def _in_specs(cfg: KCfg):
    T = cfg.T
    return {
        # q/k transposed slices (rows = head dims, fp16). *s = sigma-permuted
        # rows (rotation pairing partner), so rope is all aligned TT ops.
        "qA1": (512, T), "qB1s": (512, T),
        "qA2": (1024, T), "qB2s": (1024, T),
        "kA1": (512, T),
        "kA2": (1024, T),
        "kB1": (256, T), "kB1s": (256, T),
        # v slices, natural [T, d] layout
        "vA1": (T, 512), "vA2": (T, 1024), "vB1": (T, 256),
        # rope tables [d, T], weights+scale folded, sin sign-folded
        "ca1": (128, T), "sa1": (128, T),
        "ca2": (256, T), "sa2": (256, T),
        "cb1": (128, T), "sb1": (128, T),
        "cb2": (128, T), "sb2": (128, T),
        # v mixing: ratio rows (w0/w1, w2/w3) and 1/w den-ones columns
        "wr0": (P, 512), "wr2": (P, 256),
        "onesA": (P, 128), "onesB": (P, 128),
    }


class _Pick:
    """Static DVE-vs-Pool load balancer with cost-model-accurate weights.

    DVE: n*0.5208ns fp16 TT (2x mode), n*0.26 fp16 copy (4x), n*1.0417
    for psum/fp32 ops (+60/+125ns access). Pool: n*0.8333 flat. ACT is
    reserved for the exps (it is the 2nd-busiest engine)."""

    def __init__(self, nc):
        self.nc = nc
        self.load = {"dve": 0.0, "pool": 0.0}

    def _eng(self, cd, cp):
        if self.load["dve"] + cd <= self.load["pool"] + cp:
            self.load["dve"] += cd
            return self.nc.vector
        self.load["pool"] += cp
        return self.nc.gpsimd

    def tt16(self, out, in0, in1, op, n):
        e = self._eng(n * 0.5208 + 60, n * 0.8333 + 25)
        e.tensor_tensor(out, in0, in1, op)

    def recip(self, out, in_, n):
        """PSUM->SBUF reciprocal: DVE only (Pool has no PSUM port)."""
        self.load["dve"] += n * 1.0417 + 125
        self.nc.vector.reciprocal(out, in_)

    def dve_psmul(self, out, in0, in1, n):
        """TT mult with one PSUM f32 operand: DVE only."""
        self.load["dve"] += n * 1.0417 + 125
        self.nc.vector.tensor_tensor(out, in0, in1, mybir.AluOpType.mult)

    def act_exp(self, out, in_, n):
        self.load["act"] = self.load.get("act", 0.0) + n * 0.8333 + 185
        self.nc.scalar.activation(out, in_, mybir.ActivationFunctionType.Exp)

    def copy_ps(self, dst, src, n, on_act):
        """PSUM->SBUF copy. Phase-aware placement: ACT idles during config A
        (PE-bound) but saturates during config B, where DVE/Pool idle --
        so A-head copies go to ACT ('copy' shares the exp table, no reload)
        and B-head copies go to DVE."""
        if on_act:
            self.load["act"] = self.load.get("act", 0.0) + n * 0.8333 + 185
            self.nc.scalar.copy(dst, src)
        else:
            self.load["dve"] += n * 1.0417 + 125
            self.nc.vector.tensor_copy(dst, src)

    def copy16(self, dst, src, n):
        e = self._eng(n * 0.26 + 60, n * 0.8333 + 25)
        e.tensor_copy(dst, src)

    def pool_fix(self, n):
        self.load["pool"] += n * 0.8333 + 25
        return self.nc.gpsimd


def build_program(cfg: KCfg = FULL):
    nc = bacc.Bacc("TRN2", target_bir_lowering=False)
    T, TK, REG, NREG = cfg.T, cfg.TK, cfg.REG, cfg.NREG
    RPB = REG // P

    D = {}
    for name, shape in _in_specs(cfg).items():
        D[name] = nc.declare_dram_parameter(name, list(shape), F16, isOutput=False)
    outT = nc.declare_dram_parameter("outT", [1024, T], F16, isOutput=True)

    mult, add = mybir.AluOpType.mult, mybir.AluOpType.add

    def subchunks(c):
        out = []
        for r in range(NREG):
            q0 = max(REG * r, P * c)
            q1 = REG * (r + 1)
            if q1 > q0:
                out.append((r, q0, q1 - q0))
        return out

    last_c = [min(TK, RPB * (r + 1)) - 1 for r in range(NREG)]

    with ExitStack() as ctx:
        tc = ctx.enter_context(tile.TileContext(nc))
        const = ctx.enter_context(tc.tile_pool(name="const", bufs=1))
        rawq = ctx.enter_context(tc.tile_pool(name="rawq", bufs=1))
        vmixp = ctx.enter_context(tc.tile_pool(name="vmix", bufs=1))
        vraw = ctx.enter_context(tc.tile_pool(name="vraw", bufs=1))
        accp = ctx.enter_context(tc.tile_pool(name="acc", bufs=1))
        x2p = ctx.enter_context(tc.tile_pool(name="x2p", bufs=X2_BUFS))
        mixp = ctx.enter_context(tc.tile_pool(name="mix", bufs=MIX_BUFS))
        scr = ctx.enter_context(tc.tile_pool(name="scr", bufs=SCR_BUFS))
        normp = ctx.enter_context(tc.tile_pool(name="norm", bufs=NORM_BUFS))
        pta = ctx.enter_context(tc.tile_pool(name="pta", bufs=PTA_BUFS))
        ptb = ctx.enter_context(tc.tile_pool(name="ptb", bufs=PTB_BUFS))

        pick = _Pick(nc)

        # DMAs are emitted in first-use order (the DMA device is serial in
        # the model): A tables -> head-0 slices -> v chunks -> later heads ->
        # B-only tables/slices. Raw q/k persistent tiles are filled by
        # per-head slice DMAs so head 0's data lands first.
        onesA = const.tile([P, P], F16)
        onesB = const.tile([P, P], F16)
        nc.sync.dma_start(out=onesA, in_=D["onesA"][:, :])
        nc.sync.dma_start(out=onesB, in_=D["onesB"][:, :])
        tabs = {}

        def load_tab(nm):
            rows = _in_specs(cfg)[nm][0]
            tl = const.tile([P, rows // P, T], F16, name=nm, tag=nm)
            tabs[nm] = tl
            nc.sync.dma_start(out=tl, in_=D[nm].rearrange("(c p) t -> p c t", p=P))

        wrs = {}

        def load_wrs():
            for nm in ("wr0", "wr2"):
                cols = _in_specs(cfg)[nm][1]
                tl = const.tile([P, cols], F16, name=nm, tag=nm)
                wrs[nm] = tl
                nc.sync.dma_start(out=tl, in_=D[nm][:, :])

        raw = {}
        for nm in ("qA1", "qB1s", "kA1", "kB1", "kB1s"):
            rows = _in_specs(cfg)[nm][0]
            raw[nm] = rawq.tile([P, rows // P, T], F16, name=nm, tag=nm)
        for nm in ("qA1s", "kA1s"):  # device-built sigma64 copies
            raw[nm] = rawq.tile([P, 4, T], F16, name=nm, tag=nm)

        def emit_sigma64(nm, h):
            s_, d_ = raw[nm[:3]], raw[nm]
            pick.copy16(d_[0:64, h, :], s_[64:128, h, :], 1024)
            pick.copy16(d_[64:128, h, :], s_[0:64, h, :], 1024)

        def load_raw_slice(nm, i):
            nc.sync.dma_start(out=raw[nm][:, i, :],
                              in_=D[nm][P * i:P * (i + 1), :])

        def load_x2(dram, pair, tag):
            tl = x2p.tile([P, 2, T], F16, tag=tag, name=tag)
            for i in range(2):  # split halves: part 0 usable earlier
                nc.sync.dma_start(
                    out=tl[:, i, :],
                    in_=dram[P * (2 * pair + i):P * (2 * pair + i + 1), :])
            return tl

        # head-0 loads, interleaved with the tables each mix op needs so the
        # first products can start ~4us in; wrs (v-mix weights) come last
        q2_0 = x2p.tile([P, 2, T], F16, tag="x2a", name="x2a")
        k2_0 = x2p.tile([P, 2, T], F16, tag="x2b", name="x2b")
        nc.sync.dma_start(out=q2_0[:, 0, :], in_=D["qA2"][0:P, :])
        load_tab("ca2")
        nc.sync.dma_start(out=q2_0[:, 1, :], in_=D["qA2"][P:2 * P, :])
        load_tab("sa2")
        nc.sync.dma_start(out=k2_0[:, 0, :], in_=D["kA2"][0:P, :])
        nc.sync.dma_start(out=k2_0[:, 1, :], in_=D["kA2"][P:2 * P, :])
        load_raw_slice("kA1", 0)
        load_raw_slice("qA1", 0)
        load_tab("ca1")
        load_tab("sa1")
        load_wrs()

        # v tiles; vA2 is DMA'd straight into vmixA and scaled in place.
        # Chunk DMAs are interleaved with head-1's loads further down so the
        # serial DMA queue tracks consumption order.
        vmixA = vmixp.tile([P, TK, 4, 256], F16)
        vmixB = vmixp.tile([P, TK, 4, 128], F16)
        vB1r = vraw.tile([P, TK, 256], F16)

        def load_v_chunk(c):
            nc.sync.dma_start(out=vmixA[:, c],
                              in_=D["vA2"][P * c:P * (c + 1), :])
            nc.sync.dma_start(out=vmixB[:, c],
                              in_=D["vA1"][P * c:P * (c + 1), :])

        # pt is softmax-scale-invariant, so w1/w3 live in the den "ones"
        # columns: vmixA = vA2 + (w0/w1)*vA1 pad; vmixB = vA1 + (w2/w3)*vB1.
        # vmixA's ratio-mult reads vmixB (raw vA1) BEFORE emit_vmixB's add
        # modifies its first 64 cols per kv -- emission order guarantees it.

        def emit_vmixA():
            for c in range(TK):
                vt1 = scr.tile([P, 4, 128], F16, tag="mt2")
                pick.tt16(vt1, vmixB[:, c, :], wrs["wr0"], mult, 512)
                pick.tt16(vmixA[:, c, :, 0:128], vmixA[:, c, :, 0:128], vt1,
                          add, 512)

        def emit_vmixB():
            for c in range(TK):
                vt2 = scr.tile([P, 4, 64], F16, tag="mu2")
                pick.tt16(vt2, vB1r[:, c, :], wrs["wr2"], mult, 256)
                pick.tt16(vmixB[:, c, :, 0:64], vmixB[:, c, :, 0:64], vt2,
                          add, 256)

        outacc = accp.tile([P, 8, T], F16)

        def mix_A_dc0(dst, x1, x1s, x2):
            """dst[:,0,:] = (x2_0*c2_0 + x2_1*s2_0) + (x1*c1 + x1s*s1)."""
            t = scr.tile([P, T], F16, tag="mt")
            u = scr.tile([P, T], F16, tag="mu")
            t2 = scr.tile([P, T], F16, tag="mt2")
            u2 = scr.tile([P, T], F16, tag="mu2")
            pick.tt16(t, x2[:, 0, :], tabs["ca2"][:, 0, :], mult, 1024)
            pick.tt16(u, x2[:, 1, :], tabs["sa2"][:, 0, :], mult, 1024)
            pick.tt16(t2, x1, tabs["ca1"][:, 0, :], mult, 1024)
            pick.tt16(u2, x1s, tabs["sa1"][:, 0, :], mult, 1024)
            pick.tt16(t, t, u, add, 1024)
            pick.tt16(t2, t2, u2, add, 1024)
            pick.tt16(dst[:, 0, :], t, t2, add, 1024)

        def mix_A_dc1(dst, x2):
            """dst[:,1,:] = x2_1*c2_1 + x2_0*s2_1."""
            t = scr.tile([P, T], F16, tag="mt")
            u = scr.tile([P, T], F16, tag="mu")
            pick.tt16(t, x2[:, 1, :], tabs["ca2"][:, 1, :], mult, 1024)
            pick.tt16(u, x2[:, 0, :], tabs["sa2"][:, 1, :], mult, 1024)
            pick.tt16(dst[:, 1, :], t, u, add, 1024)

        def mix_B_pair(dst, x2, x2s, x1, x1s):
            """dst [P,2,T]: B-heads (2j,2j+1) rope mix.
            x2/x2s [P,2,T] d128 nat/sigma64; x1/x1s [P,T] packed d64 pair."""
            for hh in range(2):
                t2 = scr.tile([P, T], F16, tag="mt")
                u2 = scr.tile([P, T], F16, tag="mu")
                pick.tt16(t2, x2[:, hh, :], tabs["cb2"][:, 0, :], mult, 1024)
                pick.tt16(u2, x2s[:, hh, :], tabs["sb2"][:, 0, :], mult, 1024)
                pick.tt16(dst[:, hh, :], t2, u2, add, 1024)
            # packed d64 part for both heads
            t = scr.tile([P, T], F16, tag="mt2")
            u = scr.tile([P, T], F16, tag="mu2")
            pick.tt16(t, x1, tabs["cb1"][:, 0, :], mult, 1024)
            pick.tt16(u, x1s, tabs["sb1"][:, 0, :], mult, 1024)
            pick.tt16(t, t, u, add, 1024)
            # head 2j: rows 0:64 aligned; head 2j+1: cross-base copy first
            pick.tt16(dst[0:64, 0, :], dst[0:64, 0, :], t[0:64, :], add, 1024)
            pick.copy16(u[0:64, :], t[64:128, :], 1024)
            pick.tt16(dst[0:64, 1, :], dst[0:64, 1, :], u[0:64, :], add, 1024)

        def attn_head(qmix_dcs, kmix_dcs, vmix_ap, blks, spool, ypool, dpool,
                      wide_sT, is_b, lag=1, mid_emit=None):
            ones = onesB if is_b else onesA
            """qmix_dcs/kmix_dcs: list of [P,T] APs (one per 128-d chunk).
            vmix_ap(c, dc) -> stationary [P,128]. blks: output block ids.
            Emits PE work software-pipelined: scores(c) ... y/den(c-lag)."""
            ndc = len(qmix_dcs)
            den = dpool.tile([P, T], F32, tag="den")
            yts = [ypool.tile([P, T], F32, tag=f"yt{i}", name=f"yt{i}")
                   for i in range(ndc)]
            pending = []  # deferred (c, parts) lists, flushed `lag` behind

            def emit_norm(r):
                # region r's den/y accumulation is complete: normalize and
                # (for B) add+store now, overlapping the remaining chunks.
                # rec stays in fp16's normal range: the host pre-scales v and
                # the ones columns by 1/16 (softmax is pt-scale-invariant).
                sl = slice(REG * r, REG * (r + 1))
                rec = normp.tile([P, REG], F16, tag="rec")
                with nc.allow_low_precision(
                        reason="softmax rec in fp16; host pre-scales 1/16 "
                               "so rec stays in fp16 normal range"):
                    pick.recip(rec, den[:, sl], REG)
                for dc in range(ndc):
                    blk = blks[dc]
                    y_sb = normp.tile([P, REG], F16, tag="ysb")
                    pick.copy_ps(y_sb, yts[dc][:, sl], REG, on_act=not is_b)
                    if not is_b:
                        pick.tt16(outacc[:, blk, sl], y_sb, rec, mult, 512)
                    else:
                        tmp = normp.tile([P, REG], F16, tag="btmp")
                        pick.tt16(tmp, y_sb, rec, mult, 512)
                        pick.tt16(outacc[:, blk, sl], outacc[:, blk, sl],
                                  tmp, add, 512)
                        nc.sync.dma_start(
                            out=outT[P * blk:P * (blk + 1), sl],
                            in_=outacc[:, blk, sl])

            def flush(pend):
                c, parts = pend
                for (q0, n, pt, off) in parts:
                    r = q0 // REG
                    # the diagonal 128 cols wait on the affine mask; split
                    # them off so the bulk matmuls only depend on the exp.
                    # (not at c==0: two start=True groups in one PSUM zero
                    # region are illegal)
                    segs = [(q0, n, off)]
                    if q0 == P * c and n > P and c > 0:
                        segs = [(q0 + P, n - P, off + P), (q0, P, off)]
                    for (s0, sn, so) in segs:
                        for dc in range(ndc):
                            nc.tensor.matmul(
                                yts[dc][:, s0:s0 + sn], vmix_ap(c, dc),
                                pt[:, so:so + sn],
                                start=(c == 0), stop=(c == last_c[r]))
                        nc.tensor.matmul(den[:, s0:s0 + sn], ones,
                                         pt[:, so:so + sn],
                                         start=(c == 0), stop=(c == last_c[r]))
                for r in range(NREG):
                    if last_c[r] == c:
                        emit_norm(r)

            for c in range(TK):
                if c == (MID_CB if wide_sT else MID_C) and mid_emit is not None:
                    mid_emit()
                parts = []
                if wide_sT:
                    # one [P,T] sT + one exp for the whole chunk
                    sT = spool.tile([P, T], F32, tag="sTw")
                    pt = ptb.tile([P, T], F16, tag="ptw")
                    for (r, q0, n) in subchunks(c):
                        for dc in range(ndc):
                            nc.tensor.matmul(
                                sT[:, q0:q0 + n],
                                kmix_dcs[dc][:, P * c:P * (c + 1)],
                                qmix_dcs[dc][:, q0:q0 + n],
                                start=(dc == 0), stop=(dc == ndc - 1))
                    pick.act_exp(pt[:, P * c:], sT[:, P * c:], T - P * c)
                    parts = [(q0, n, pt, q0) for (r, q0, n) in subchunks(c)]
                    diag = (pt, P * c)
                else:
                    # [P,REG] sT + exp per subchunk
                    diag = None
                    for (r, q0, n) in subchunks(c):
                        sT = spool.tile([P, REG], F32, tag="sTn")
                        pt = pta.tile([P, REG], F16, tag="ptn")
                        for dc in range(ndc):
                            nc.tensor.matmul(
                                sT[:, 0:n],
                                kmix_dcs[dc][:, P * c:P * (c + 1)],
                                qmix_dcs[dc][:, q0:q0 + n],
                                start=(dc == 0), stop=(dc == ndc - 1))
                        pick.act_exp(pt[:, 0:n], sT[:, 0:n], n)
                        if q0 == P * c:
                            diag = (pt, 0)
                        parts.append((q0, n, pt, 0))
                # zero the masked (k>q) half of the causal diagonal block
                dpt, doff = diag
                pick.pool_fix(128).affine_select(
                    out=dpt[:, doff:doff + P], in_=dpt[:, doff:doff + P],
                    compare_op=mybir.AluOpType.is_ge, fill=0.0,
                    base=0, pattern=[[1, P]], channel_multiplier=-1)
                pending.append((c, parts))
                if len(pending) > lag:
                    flush(pending.pop(0))
            for pend in pending:
                flush(pend)

        def mix_A_head(h, q2=None, k2=None):
            # DMAs emitted here so the serial DMA queue runs in use order;
            # dc0 chains for q AND k first so the first score matmul (which
            # only needs dc0 of both) is unblocked as early as possible.
            if q2 is None:
                q2 = load_x2(D["qA2"], h, "x2a")
                load_raw_slice("qA1", h)
                k2 = load_x2(D["kA2"], h, "x2b")
                load_raw_slice("kA1", h)
            emit_sigma64("qA1s", h)
            emit_sigma64("kA1s", h)
            qmix = mixp.tile([P, 2, T], F16, tag="qmix")
            kmix = mixp.tile([P, 2, T], F16, tag="kmix")
            mix_A_dc0(qmix, raw["qA1"][:, h, :], raw["qA1s"][:, h, :], q2)
            mix_A_dc0(kmix, raw["kA1"][:, h, :], raw["kA1s"][:, h, :], k2)
            mix_A_dc1(qmix, q2)
            mix_A_dc1(kmix, k2)
            return qmix, kmix

        def mix_B_group(j):
            """kv j's kmix + the q pair for heads (2j, 2j+1)."""
            if j == 0:
                for nm in ("cb2", "sb2", "cb1", "sb1"):
                    load_tab(nm)
            if j % 2 == 0:
                load_raw_slice("kB1", j // 2)
                load_raw_slice("kB1s", j // 2)
            load_raw_slice("qB1s", j)
            kmix = mixp.tile([P, T], F16, tag="kmixB")
            t = scr.tile([P, T], F16, tag="mt2")
            u = scr.tile([P, T], F16, tag="mu2")
            pick.tt16(t, raw["kA1"][:, j, :], tabs["cb2"][:, 0, :], mult, 1024)
            pick.tt16(u, raw["kA1s"][:, j, :], tabs["sb2"][:, 0, :], mult, 1024)
            pick.tt16(kmix, t, u, add, 1024)
            # d64 part: computed once per kv pair (kB1 tile j//2), reused by
            # the odd kv via a cross-base copy
            jp, g = j // 2, j % 2
            if g == 0:
                t64 = scr.tile([P, T], F16, tag="t64")
                pick.tt16(t64, raw["kB1"][:, jp, :], tabs["cb1"][:, 0, :],
                          mult, 1024)
                pick.tt16(u, raw["kB1s"][:, jp, :], tabs["sb1"][:, 0, :],
                          mult, 1024)
                pick.tt16(t64, t64, u, add, 1024)
                st["t64"] = t64
                pick.tt16(kmix[0:64, :], kmix[0:64, :], t64[0:64, :], add, 1024)
            else:
                pick.copy16(u[0:64, :], st["t64"][64:128, :], 1024)
                pick.tt16(kmix[0:64, :], kmix[0:64, :], u[0:64, :], add, 1024)

            q2 = load_x2(D["qA2"], j, "x2a")
            q2s = load_x2(D["qB2s"], j, "x2b")
            qmixp = mixp.tile([P, 2, T], F16, tag="qmix")
            mix_B_pair(qmixp, q2, q2s,
                       raw["qA1"][:, j, :], raw["qB1s"][:, j, :])
            return qmixp, kmix

        # ============ config A (B group 0's mix overlaps A3's attn) ========
        st = {"amix": mix_A_head(0, q2_0, k2_0), "bmix": None}
        # interleave v-chunk DMAs with head 1's loads in the serial DMA queue
        load_v_chunk(0)
        load_v_chunk(1)
        q2_1 = load_x2(D["qA2"], 1, "x2a")
        load_raw_slice("qA1", 1)
        load_v_chunk(2)
        load_v_chunk(3)
        k2_1 = load_x2(D["kA2"], 1, "x2b")
        load_raw_slice("kA1", 1)
        for c in range(4, TK):
            load_v_chunk(c)
        emit_vmixA()

        with tc.tile_pool(name="spsA", bufs=2, space="PSUM") as spsA, \
             tc.tile_pool(name="ypsA", bufs=1, space="PSUM") as ypsA, \
             tc.tile_pool(name="dpsA", bufs=1, space="PSUM") as dpsA:
            for h in range(cfg.NA):
                qmix, kmix = st["amix"]

                def filler(h=h):
                    # vB1/vmixB wait until h==2 so heads 2-3's DMA bundles
                    # are not queued behind them (vmixB is first used by B0)
                    if h == 0:
                        st["amix"] = mix_A_head(1, q2_1, k2_1)
                    elif h == 1:
                        st["amix"] = mix_A_head(2)
                    elif h == 2:
                        st["amix"] = mix_A_head(3)
                        for c in range(TK):
                            nc.sync.dma_start(out=vB1r[:, c, :],
                                              in_=D["vB1"][P * c:P * (c + 1), :])
                        emit_vmixB()
                    else:
                        st["bmix"] = mix_B_group(0)

                def vap(c, dc, h=h):
                    return vmixA[:, c, h, 128 * dc:128 * (dc + 1)]

                attn_head([qmix[:, 0, :], qmix[:, 1, :]],
                          [kmix[:, 0, :], kmix[:, 1, :]],
                          vap, (2 * h, 2 * h + 1), spsA, ypsA, dpsA,
                          wide_sT=False, is_b=False, lag=LAG_A, mid_emit=filler)

        # ================= config B =================
        with tc.tile_pool(name="spsB", bufs=2, space="PSUM") as spsB, \
             tc.tile_pool(name="ypsB", bufs=1, space="PSUM") as ypsB, \
             tc.tile_pool(name="dpsB", bufs=1, space="PSUM") as dpsB:
            for j in range(4):  # kv head j serves B-heads (2j, 2j+1)
                qmixp, kmix = st["bmix"]

                def vapB(c, dc, j=j):
                    return vmixB[:, c, j, :]

                for hh in range(2):
                    b = 2 * j + hh
                    filler = None
                    if hh == 1 and j + 1 < 4:
                        def filler(j=j):
                            st["bmix"] = mix_B_group(j + 1)
                    attn_head([qmixp[:, hh, :]], [kmix], vapB, (b,),
                              spsB, ypsB, dpsB, wide_sT=True, is_b=True,
                              lag=LAG_B, mid_emit=filler)

    nc.compile()
    return nc


# ---------------------------------------------------------------------------
# Host side
# ---------------------------------------------------------------------------

def _rope_tab(pos, d, f):
    """Rope tables [d, T]: (f*cos, signed f*sin); sign folded so that
    out[j] = x[j]*c[j] + x[sigma(j)]*s[j] with sigma the half-swap."""
    inv = 1.0 / (10000.0 ** (np.arange(0, d, 2, dtype=np.float32) / d))
    ang = inv[:, None] * pos[None, :].astype(np.float32)      # [d/2, T]
    ang = np.concatenate([ang, ang], 0)                        # [d, T]
    c = (f * np.cos(ang)).astype(np.float32)
    s = (f * np.sin(ang)).astype(np.float32)
    s[: d // 2] *= -1.0
    return c, s


def _sigma(x, half, group):
    """Swap half-blocks of `half` rows within each `group`-row group."""
    r = x.reshape(-1, 2, half, x.shape[-1])
    assert group == 2 * half
    return np.ascontiguousarray(r[:, ::-1].reshape(x.shape))


def make_core_inputs(q, k, v, pos, weights, s, cfg: KCfg = FULL):
    """q,k,v: [T, 2048] fp32 for one batch; returns per-core input dict."""
    f16 = lambda a: np.ascontiguousarray(a, dtype=np.float16)
    qT1 = np.ascontiguousarray(q[:, 512 * s:512 * s + 512].T)
    qT2 = np.ascontiguousarray(q[:, 1024 * s:1024 * s + 1024].T)
    kT1 = np.ascontiguousarray(k[:, 512 * s:512 * s + 512].T)
    kT2 = np.ascontiguousarray(k[:, 1024 * s:1024 * s + 1024].T)
    kB1 = np.ascontiguousarray(k[:, 256 * s:256 * s + 256].T)
    arrs = {
        "qA1": f16(qT1),
        "qB1s": f16(_sigma(qT1, 32, 64)),
        "qA2": f16(qT2), "qB2s": f16(_sigma(qT2, 64, 128)),
        "kA1": f16(kT1),
        "kA2": f16(kT2),
        "kB1": f16(kB1), "kB1s": f16(_sigma(kB1, 32, 64)),
        "vA1": f16(v[:, 512 * s:512 * s + 512] / 16.0),
        "vA2": f16(v[:, 1024 * s:1024 * s + 1024] / 16.0),
        "vB1": f16(v[:, 256 * s:256 * s + 256] / 16.0),
    }
    fA = math.sqrt(1.0 / 16.0)
    fB = math.sqrt(1.0 / math.sqrt(128.0))
    ca1, sa1 = _rope_tab(pos, 128, fA * float(weights[0]))
    ca2, sa2 = _rope_tab(pos, 256, fA * float(weights[1]))
    cb1h, sb1h = _rope_tab(pos, 64, fB * float(weights[2]))
    cb2, sb2 = _rope_tab(pos, 128, fB * float(weights[3]))
    arrs.update({
        "ca1": f16(ca1), "sa1": f16(sa1),
        "ca2": f16(ca2), "sa2": f16(sa2),
        "cb1": f16(np.vstack([cb1h, cb1h])), "sb1": f16(np.vstack([sb1h, sb1h])),
        "cb2": f16(cb2), "sb2": f16(sb2),
        # pt is scale-invariant under softmax: fold w1 (w3) into the den
        # "ones" columns and keep only the w0/w1 (w2/w3) ratio on the v1 add
        "wr0": f16(np.full((P, 512), float(weights[0] / weights[1]))),
        "wr2": f16(np.full((P, 256), float(weights[2] / weights[3]))),
        "onesA": f16(np.full((P, 128), float(1.0 / (16.0 * weights[1])))),
        "onesB": f16(np.full((P, 128), float(1.0 / (16.0 * weights[3])))),
    })
    return arrs


_PROGRAM_CACHE = {}
TRACE = False
LAST_RESULT = None
NEG = -1e9


def kernel(q_m, k_m, v_m, weights, attention_mask, position_ids):
    global LAST_RESULT
    from concourse.bass_utils import run_bass_kernel_spmd

    cfg = FULL
    q_m = np.asarray(q_m, np.float32)
    k_m = np.asarray(k_m, np.float32)
    v_m = np.asarray(v_m, np.float32)
    weights = np.asarray(weights, np.float32)
    attention_mask = np.asarray(attention_mask, np.float32)
    position_ids = np.asarray(position_ids)
    B, T, H = q_m.shape

    # the device program hardcodes the causal structure; verify it holds
    causal = np.where(np.tril(np.ones((T, T), bool)), 0.0, NEG).astype(np.float32)
    for b in range(B):
        assert np.array_equal(attention_mask[b, 0], causal), "non-causal mask"

    if "nc" not in _PROGRAM_CACHE:
        _PROGRAM_CACHE["nc"] = build_program(cfg)
    nc = _PROGRAM_CACHE["nc"]

    in_maps = []
    for b in range(B):
        for s in range(2):
            in_maps.append(make_core_inputs(
                q_m[b], k_m[b], v_m[b], position_ids[b], weights, s, cfg))
    res = run_bass_kernel_spmd(nc, in_maps, list(range(8)), trace=TRACE)
    LAST_RESULT = res
    out = np.zeros((B, T, H), np.float32)
    for b in range(B):
        for s in range(2):
            out[b, :, 1024 * s:1024 * s + 1024] = \
                res.results[2 * b + s]["outT"].astype(np.float32).T
    return out
